# revision 1
# baseline (speedup 1.0000x reference)
"""DRew-GCN forward on 8 Trainium2 NeuronCores.

Node-partitioned across cores (block-contiguous, graph-aligned). Edges live
with the core owning dst; per hop they are grouped into 128-node dst windows
(uniform tile capacities across cores so one SPMD program serves all 8).
Scatter-add = one-hot matmul into PSUM (lhsT = gathered msg rows, rhs =
Q[e,j] = coef_src[e] * (dstloc[e]==j)). Rows are fetched by dma_gather from
replicated bf16 node tables refreshed per layer with an AllGather. The GCN
projection W is applied after aggregation (linearity), so gathers move only
raw 128-dim rows.
"""
import sys
import numpy as np

if '/opt/trn_rl_repo' not in sys.path:
    sys.path.insert(0, '/opt/trn_rl_repo')


class Cfg:
    def __init__(self, N=50000, G=500, IN_DIM=32, HID=128, OUT=10,
                 L=5, E_K=400000, NCORES=8, CH=32):
        self.N, self.G, self.IN_DIM, self.HID, self.OUT = N, G, IN_DIM, HID, OUT
        self.L, self.E_K, self.NCORES, self.CH = L, E_K, NCORES, CH
        self.NPG = N // G                       # nodes per graph
        assert N % G == 0
        self.GPC = -(-G // NCORES)              # graphs per core (ceil)
        self.BLK = self.GPC * self.NPG          # real nodes per core (last less)
        self.NLOC = 128 * (-(-self.BLK // 128))
        self.W = self.NLOC // 128
        self.TBL = NCORES * self.NLOC
        self.HALF = self.TBL // 2
        assert self.HALF < 32768
        self.GPAD = 128 * (-(-G // 128))
        self.NCONV = L * (L + 1) // 2
        self.SMAX = 64                          # pooling slice tile width
        self.SMAXR = self.GPC                   # real slices per core
        assert self.SMAXR <= self.SMAX

    def core_lo(self, c): return min(self.BLK * c, self.N)
    def core_hi(self, c): return min(self.BLK * (c + 1), self.N)

    def row_of(self, n):
        c = np.minimum(n // self.BLK, self.NCORES - 1)
        return self.NLOC * c + (n - self.BLK * c)


def build_plan(cfg, k_edge_index, batch):
    NC = cfg.NCORES
    hops = []
    for k in range(1, cfg.L + 1):
        src = np.asarray(k_edge_index[0, (k - 1) * cfg.E_K: k * cfg.E_K], np.int64)
        dst = np.asarray(k_edge_index[1, (k - 1) * cfg.E_K: k * cfg.E_K], np.int64)
        per_core = []
        for c in range(NC):
            lo_n, hi_n = cfg.core_lo(c), cfg.core_hi(c)
            m = (dst >= lo_n) & (dst < hi_n)
            es, ed = src[m], dst[m]
            selfn = np.arange(lo_n, hi_n, dtype=np.int64)
            es = np.concatenate([es, selfn]); ed = np.concatenate([ed, selfn])
            erow = cfg.row_of(es)
            dloc = ed - lo_n
            half = (erow % 2).astype(np.int64)
            per_core.append((erow, dloc, half, dloc // 128))
        T = np.zeros((cfg.W, 2), np.int64)
        for c in range(NC):
            _, _, half, w = per_core[c]
            for h in (0, 1):
                cnt = np.bincount(w[half == h], minlength=cfg.W)
                T[:, h] = np.maximum(T[:, h], -(-cnt // 128))
        TL, TH = int(T[:, 0].sum()), int(T[:, 1].sum())
        Ttot = TL + TH
        tile_w = np.concatenate([np.repeat(np.arange(cfg.W), T[:, 0]),
                                 np.repeat(np.arange(cfg.W), T[:, 1])])
        # first/last tile flags per segment and per window (across whole hop)
        seg_first = np.zeros(Ttot, bool); seg_last = np.zeros(Ttot, bool)
        pos = 0
        seg_off = np.zeros((cfg.W, 2), np.int64)
        for h in (0, 1):
            for w in range(cfg.W):
                seg_off[w, h] = pos
                if T[w, h]:
                    seg_first[pos] = True
                    seg_last[pos + T[w, h] - 1] = True
                    pos += int(T[w, h])
        w_first = np.zeros(Ttot, bool); w_last = np.zeros(Ttot, bool)
        for w in range(cfg.W):
            ts = np.nonzero(tile_w == w)[0]
            if len(ts):
                w_first[ts[0]] = True
                w_last[ts[-1]] = True
        idx_all = np.zeros((NC, Ttot * 128), np.int16)
        dstrel_all = np.full((NC, Ttot * 128), -1.0, np.float32)
        for c in range(NC):
            erow, dloc, half, w = per_core[c]
            for h in (0, 1):
                sel = half == h
                ws, rows, dl = w[sel], erow[sel], dloc[sel]
                order = np.argsort(ws, kind='stable')
                ws, rows, dl = ws[order], rows[order], dl[order]
                cnts = np.bincount(ws, minlength=cfg.W)
                starts = seg_off[:, h] * 128
                grp0 = np.concatenate([[0], np.cumsum(cnts)[:-1]])
                pos_in = np.arange(len(ws)) - np.repeat(grp0, cnts)
                p = starts[ws] + pos_in
                idx_all[c, p] = (rows >> 1).astype(np.int16)
                dstrel_all[c, p] = (dl - 128 * ws).astype(np.float32)
        hops.append(dict(T=T, TL=TL, TH=TH, Ttot=Ttot, tile_w=tile_w,
                         seg_first=seg_first, seg_last=seg_last,
                         w_first=w_first, w_last=w_last,
                         idx=idx_all, dstrel=dstrel_all))
    P = np.zeros((NC, cfg.SMAX, cfg.GPAD), np.float32)
    for c in range(NC):
        lo_n, hi_n = cfg.core_lo(c), cfg.core_hi(c)
        gbase = lo_n // cfg.NPG
        for s in range((hi_n - lo_n) // cfg.NPG):
            P[c, s, gbase + s] = 1.0
    b = np.asarray(batch, np.int64)
    cnt = np.bincount(b, minlength=cfg.G)
    assert (cnt == cfg.NPG).all() and (np.sort(b) == b).all(), \
        "batch must be contiguous-uniform"
    key = tuple(int(h['Ttot']) for h in hops)
    return dict(hops=hops, P=P, key=key)


def build_bass(cfg, plan):
    import concourse.bacc as bacc
    import concourse.mybir as mybir
    from concourse.tile import TileContext
    from concourse.library_config import mlp as mlp_lib

    f32, bf16, i16 = mybir.dt.float32, mybir.dt.bfloat16, mybir.dt.int16
    Alu = mybir.AluOpType
    Act = mybir.ActivationFunctionType
    AX = mybir.AxisListType.X
    NC, L, W, CH = cfg.NCORES, cfg.L, cfg.W, cfg.CH
    HID, GPAD = cfg.HID, cfg.GPAD
    RG = [list(range(NC))]

    nc = bacc.Bacc("TRN2", num_devices=NC)

    xT = nc.dram_tensor("xT", [cfg.IN_DIM, cfg.NLOC], f32, kind="ExternalInput")
    idx_d, dsr_d = [], []
    for k in range(1, L + 1):
        hp = plan['hops'][k - 1]
        idx_d.append(nc.dram_tensor(f"idx{k}", [128, hp['Ttot'] * 8], i16,
                                    kind="ExternalInput"))
        dsr_d.append(nc.dram_tensor(f"dsr{k}", [128, hp['Ttot']], f32,
                                    kind="ExternalInput"))
    iota_d = nc.dram_tensor("iota", [128, 128], f32, kind="ExternalInput")
    ident_d = nc.dram_tensor("ident", [128, 128], f32, kind="ExternalInput")
    P_d = nc.dram_tensor("P", [cfg.SMAX, GPAD], f32, kind="ExternalInput")
    embWT_d = nc.dram_tensor("embWT", [cfg.IN_DIM, HID], f32, kind="ExternalInput")
    embB_d = nc.dram_tensor("embB", [1, HID], f32, kind="ExternalInput")
    convWT_d = nc.dram_tensor("convWT", [cfg.NCONV, HID, HID], bf16,
                              kind="ExternalInput")
    convB_d = nc.dram_tensor("convB", [cfg.NCONV, HID], f32, kind="ExternalInput")
    kinv_d = nc.dram_tensor("kinv", [cfg.NCONV, 1], f32, kind="ExternalInput")
    r1WT_d = nc.dram_tensor("r1WT", [3 * HID, 192], f32, kind="ExternalInput")
    r1B_d = nc.dram_tensor("r1B", [192, 1], f32, kind="ExternalInput")
    r2WT_d = nc.dram_tensor("r2WT", [192, cfg.OUT], f32, kind="ExternalInput")
    r2B_d = nc.dram_tensor("r2B", [cfg.OUT, 1], f32, kind="ExternalInput")
    y_d = nc.dram_tensor("y", [cfg.OUT, GPAD], f32, kind="ExternalOutput")
    dbg_d = None
    if getattr(cfg, 'debug', False):
        T1 = plan['hops'][0]['Ttot']
        dbg_d = nc.dram_tensor("dbg", [128, cfg.L * W + T1], f32,
                               kind="ExternalOutput")

    tables = [nc.dram_tensor(f"tbl{j}", [cfg.TBL, HID], bf16, kind="Internal",
                             addr_space="Shared") for j in range(L)]
    tdinv = nc.dram_tensor("tdinv", [cfg.TBL, HID], bf16, kind="Internal")
    hin = nc.dram_tensor("hin", [cfg.NLOC, HID], bf16, kind="Internal")
    din_in = nc.dram_tensor("din_in", [cfg.NLOC, 8], bf16, kind="Internal")
    din_out = nc.dram_tensor("din_out", [cfg.TBL, 8], bf16, kind="Internal",
                             addr_space="Shared")
    ps_in = [nc.dram_tensor(f"pool_in{i}", [128, GPAD], f32, kind="Internal")
             for i in range(2)]
    ps_out = [nc.dram_tensor(f"pool_out{i}", [128, GPAD], f32, kind="Internal",
                             addr_space="Shared") for i in range(2)]

    hview = hin.rearrange("(w j) f -> j w f", j=128)
    dinview = din_in.rearrange("(w j) c -> j w c", j=128)

    with TileContext(nc) as tc:
        nc.gpsimd.load_library(mlp_lib)
        with tc.tile_pool(name="const", bufs=1) as constp, \
             tc.tile_pool(name="persist", bufs=1) as pers, \
             tc.tile_pool(name="io", bufs=2) as iop, \
             tc.tile_pool(name="msg", bufs=3) as msgp, \
             tc.tile_pool(name="qp", bufs=4) as qp, \
             tc.tile_pool(name="agg", bufs=3, space="PSUM") as aggp, \
             tc.tile_pool(name="outp", bufs=2, space="PSUM") as outp, \
             tc.tile_pool(name="smallps", bufs=2, space="PSUM") as smallp:

            iota = constp.tile([128, 128], f32)
            nc.sync.dma_start(iota[:], iota_d[:])
            ident = constp.tile([128, 128], f32)
            nc.sync.dma_start(ident[:], ident_d[:])
            ones_bf = constp.tile([128, 1], bf16)
            nc.vector.memset(ones_bf[:], 1.0)
            ones_row = constp.tile([1, 128], f32)
            nc.vector.memset(ones_row[:], 1.0)

            xk = pers.tile([128, W, HID], f32)
            dinv = [pers.tile([128, W], f32, tag=f"dinv{k}", name=f"dinv{k}")
                    for k in range(L)]
            coef = [pers.tile([128, plan['hops'][k]['Ttot']], f32,
                              tag=f"coef{k}", name=f"coef{k}") for k in range(L)]
            dsr = [pers.tile([128, plan['hops'][k]['Ttot']], f32,
                             tag=f"dsrs{k}", name=f"dsrs{k}") for k in range(L)]
            h5T = pers.tile([128, W, 128], f32)
            for k in range(L):
                nc.sync.dma_start(dsr[k][:], dsr_d[k][:])

            # ---- Phase A: h0 = x @ embW^T + emb_b
            embWT = constp.tile([cfg.IN_DIM, HID], f32)
            nc.sync.dma_start(embWT[:], embWT_d[:])
            embB = constp.tile([1, HID], f32)
            nc.sync.dma_start(embB[:], embB_d[:])
            xTs = pers.tile([cfg.IN_DIM, cfg.NLOC], f32)
            nc.sync.dma_start(xTs[:], xT[:])
            bias_ps = smallp.tile([128, 128], f32, tag="smallt")
            nc.tensor.matmul(bias_ps[:], ones_row[:], embB[:])
            embB_bc = constp.tile([128, 128], f32)
            nc.vector.tensor_copy(embB_bc[:], bias_ps[:])
            h0bf = pers.tile([128, W, HID], bf16, tag="hstage")
            for w in range(W):
                hps = outp.tile([128, HID], f32, tag="ops")
                nc.tensor.matmul(hps[:], xTs[:, w * 128:(w + 1) * 128], embWT[:])
                nc.vector.tensor_tensor(h0bf[:, w, :], hps[:], embB_bc[:], Alu.add)
            nc.sync.dma_start(hview[:, :, :], h0bf[:])
            nc.gpsimd.collective_compute("AllGather", Alu.bypass,
                                         replica_groups=RG,
                                         ins=[hin[:]], outs=[tables[0][:]])

            lvl = getattr(cfg, 'lvl', 99)
            # ---- Phase B: degree -> dinv per hop
            for k in range(1, L + 1) if lvl >= 2 else []:
                hp = plan['hops'][k - 1]
                degsb = iop.tile([128, W], f32, tag="degsb")
                nc.vector.memset(degsb[:], 0.0)
                dstate = {}
                for t in range(hp['Ttot']):
                    w = int(hp['tile_w'][t])
                    q0 = qp.tile([128, 128], bf16, tag="q")
                    nc.vector.tensor_scalar(q0[:], iota[:],
                                            dsr[k - 1][:, t:t + 1], None,
                                            Alu.is_equal)
                    if hp['seg_first'][t]:
                        dstate['d'] = aggp.tile([128, 1], f32, tag="agg",
                                                name="degt")
                    nc.tensor.matmul(dstate['d'][:], q0[:], ones_bf[:],
                                     start=bool(hp['seg_first'][t]),
                                     stop=bool(hp['seg_last'][t]))
                    if hp['seg_last'][t]:
                        nc.vector.scalar_tensor_tensor(
                            degsb[:, w:w + 1], dstate['d'][:], 1.0,
                            degsb[:, w:w + 1], Alu.mult, Alu.add)
                nc.vector.tensor_scalar_max(degsb[:], degsb[:], 1.0)
                rcp = iop.tile([128, W], f32, tag="rcp")
                nc.vector.reciprocal(rcp[:], degsb[:])
                nc.scalar.activation(dinv[k - 1][:], rcp[:], Act.Sqrt)
                dvbf = iop.tile([128, W], bf16, tag="dvbf")
                nc.vector.tensor_copy(dvbf[:], dinv[k - 1][:])
                nc.sync.dma_start(dinview[:, :, k - 1:k], dvbf[:, :, None])
            if lvl >= 2:
                nc.gpsimd.collective_compute("AllGather", Alu.bypass,
                                             replica_groups=RG,
                                             ins=[din_in[:]], outs=[din_out[:]])
            if dbg_d is not None and lvl >= 2:
                for k in range(L):
                    nc.sync.dma_start(dbg_d[:, k * W:(k + 1) * W], dinv[k][:])
            nrow_t = cfg.TBL // 128
            if lvl >= 2:
                dexp = constp.tile([128, nrow_t, 8], bf16)
                nc.sync.dma_start(dexp[:],
                                  din_out.rearrange("(a b) c -> b a c", b=128))
                nc.sync.dma_start(
                    tdinv.rearrange("(a b) c -> b a c", b=128)[:, :, 0:8],
                    dexp[:])

            def gather_stream(k, table, on_chunk):
                hp = plan['hops'][k - 1]
                for (h, lim0, lim1) in ((0, 0, hp['TL']),
                                        (1, hp['TL'], hp['Ttot'])):
                    t0 = lim0
                    while t0 < lim1:
                        n_t = min(CH, lim1 - t0)
                        islab = iop.tile([128, CH * 8], i16, tag="islab")
                        nc.sync.dma_start(islab[:, :n_t * 8],
                                          idx_d[k - 1][:, t0 * 8:(t0 + n_t) * 8])
                        buf = msgp.tile([128, CH, HID], bf16, tag="gbuf")
                        tv = table.rearrange("(r two) f -> r (two f)", two=2)
                        src = tv[:, 0:HID] if h == 0 else tv[:, HID:2 * HID]
                        nc.gpsimd.dma_gather(buf[:, 0:n_t, :], src,
                                             islab[:, :n_t * 8],
                                             n_t * 128, n_t * 128, HID,
                                             elem_step=2 * HID,
                                             single_packet=False)
                        on_chunk(t0, n_t, buf)
                        t0 += n_t

            # ---- Phase C: coef = dinv_k[src] per edge
            for k in range(1, L + 1) if lvl >= 3 else []:
                def c_chunk(t0, n_t, buf, k=k):
                    nc.vector.tensor_copy(
                        coef[k - 1][:, t0:t0 + n_t],
                        buf[:, 0:n_t, k - 1:k].rearrange("p a b -> p (a b)"))
                gather_stream(k, tdinv, c_chunk)
            if dbg_d is not None and lvl >= 3:
                nc.sync.dma_start(dbg_d[:, L * W:], coef[0][:])

            # ---- Phase D: layers
            for l in range(min(L, max(0, lvl - 3))):
                nc.vector.memset(xk[:], 0.0)
                for k in range(l + 1, 0, -1):
                    ci = l * (l + 1) // 2 + (k - 1)
                    hp = plan['hops'][k - 1]
                    tbl_j = tables[l - k + 1]
                    wt = iop.tile([128, HID], bf16, tag="wt")
                    nc.sync.dma_start(wt[:], convWT_d[ci, :, :])
                    dok = iop.tile([128, W], f32, tag="dok")
                    nc.vector.tensor_scalar_mul(dok[:], dinv[k - 1][:], 1.0 / k)
                    state = {}

                    def d_chunk(t0, n_t, buf, k=k, hp=hp, wt=wt, dok=dok,
                                state=state):
                        for i in range(n_t):
                            t = t0 + i
                            w = int(hp['tile_w'][t])
                            q = qp.tile([128, 128], bf16, tag="q")
                            nc.vector.tensor_scalar(
                                q[:], iota[:], dsr[k - 1][:, t:t + 1],
                                coef[k - 1][:, t:t + 1], Alu.is_equal, Alu.mult)
                            if hp['seg_first'][t]:
                                state['agg'] = aggp.tile(
                                    [128, 128], f32, tag="agg", name="aggt")
                            nc.tensor.matmul(state['agg'][:], buf[:, i, :], q[:],
                                             start=bool(hp['seg_first'][t]),
                                             stop=bool(hp['seg_last'][t]))
                            if hp['seg_last'][t]:
                                at = qp.tile([128, 128], bf16, tag="at")
                                nc.scalar.copy(at[:], state['agg'][:])
                                ops = outp.tile([128, 128], f32, tag="ops")
                                nc.tensor.matmul(ops[:], at[:], wt[:])
                                nc.vector.scalar_tensor_tensor(
                                    xk[:, w, :], ops[:], dok[:, w:w + 1],
                                    xk[:, w, :], Alu.mult, Alu.add)
                    gather_stream(k, tbl_j, d_chunk)
                # bias: xk += bcast(sum_ci convB[ci] / k)
                c0 = l * (l + 1) // 2
                bsc = iop.tile([l + 1, HID], f32, tag="bsc")
                nc.sync.dma_start(bsc[:], convB_d[c0:c0 + l + 1, :])
                kv = iop.tile([l + 1, 1], f32, tag="kv")
                nc.sync.dma_start(kv[:], kinv_d[c0:c0 + l + 1, :])
                bscl = iop.tile([l + 1, HID], f32, tag="bscl")
                nc.vector.tensor_scalar_mul(bscl[:], bsc[:], kv[:, 0:1])
                ones_col = iop.tile([l + 1, 1], f32, tag="onescol")
                nc.vector.memset(ones_col[:], 1.0)
                brow_ps = smallp.tile([1, HID], f32, tag="smallt")
                nc.tensor.matmul(brow_ps[:], ones_col[:], bscl[:])
                brow = iop.tile([1, HID], f32, tag="brows")
                nc.vector.tensor_copy(brow[:], brow_ps[:])
                bbc_ps = smallp.tile([128, HID], f32, tag="smallt")
                nc.tensor.matmul(bbc_ps[:], ones_row[:], brow[:])
                bbc = iop.tile([128, HID], f32, tag="bbcs")
                nc.vector.tensor_copy(bbc[:], bbc_ps[:])
                nc.vector.tensor_tensor(xk[:], xk[:],
                                        bbc[:, None, :].broadcast_to(
                                            [128, W, HID]), Alu.add)
                if l < L - 1:
                    hbf = pers.tile([128, W, HID], bf16, tag="hstage")
                    nc.scalar.activation(hbf[:], xk[:], Act.Relu)
                    nc.sync.dma_start(hview[:, :, :], hbf[:])
                    nc.gpsimd.collective_compute(
                        "AllGather", Alu.bypass, replica_groups=RG,
                        ins=[hin[:]], outs=[tables[l + 1][:]])
                else:
                    nc.scalar.activation(xk[:], xk[:], Act.Relu)

            if lvl < 9:
                zsb = iop.tile([cfg.OUT, GPAD], f32, tag="ysb", name="zsb")
                nc.vector.memset(zsb[:], 0.0)
                nc.sync.dma_start(y_d[:], zsb[:])
            else:
                # ---- Phase E: pooling
                for w in range(W):
                    tp = outp.tile([128, 128], f32, tag="ops")
                    nc.tensor.transpose(tp[:], xk[:, w, :], ident[:])
                    nc.vector.tensor_copy(h5T[:, w, :], tp[:])
                h5flat = h5T[:].rearrange("p w j -> p (w j)")
                ssum_l = iop.tile([128, cfg.SMAX], f32, tag="ssum_l")
                smax_l = iop.tile([128, cfg.SMAX], f32, tag="smax_l")
                nc.vector.memset(ssum_l[:], 0.0)
                nc.vector.memset(smax_l[:], 0.0)
                for s in range(cfg.SMAXR):
                    sl = h5flat[:, s * cfg.NPG:(s + 1) * cfg.NPG]
                    nc.vector.tensor_reduce(ssum_l[:, s:s + 1], sl, axis=AX,
                                            op=Alu.add)
                    nc.vector.tensor_reduce(smax_l[:, s:s + 1], sl, axis=AX,
                                            op=Alu.max)
                Ps = constp.tile([cfg.SMAX, GPAD], f32)
                nc.sync.dma_start(Ps[:], P_d[:])
                ssum_t = iop.tile([cfg.SMAX, 128], f32, tag="ssum_t")
                smax_t = iop.tile([cfg.SMAX, 128], f32, tag="smax_t")
                for tsb, tsl in ((ssum_t, ssum_l), (smax_t, smax_l)):
                    tps2 = smallp.tile([cfg.SMAX, 128], f32, tag="smallt",
                                       name="tps2")
                    nc.tensor.transpose(tps2[:], tsl[:], ident[:])
                    nc.vector.tensor_copy(tsb[:], tps2[:])
                for i, tsrc in enumerate((ssum_t, smax_t)):
                    pps = outp.tile([128, GPAD], f32, tag="ops")
                    nc.tensor.matmul(pps[:], tsrc[:], Ps[:])
                    psb = iop.tile([128, GPAD], f32, tag=f"psb{i}")
                    nc.vector.tensor_copy(psb[:], pps[:])
                    nc.sync.dma_start(ps_in[i][:], psb[:])
                nc.gpsimd.collective_compute("AllReduce", Alu.add, replica_groups=RG,
                                             ins=[ps_in[0][:]], outs=[ps_out[0][:]])
                nc.gpsimd.collective_compute("AllReduce", Alu.max, replica_groups=RG,
                                             ins=[ps_in[1][:]], outs=[ps_out[1][:]])
                ssumR = pers.tile([128, GPAD], f32)
                nc.sync.dma_start(ssumR[:], ps_out[0][:])
                smaxR = pers.tile([128, GPAD], f32)
                nc.sync.dma_start(smaxR[:], ps_out[1][:])
                smeanR = pers.tile([128, GPAD], f32)
                nc.vector.tensor_scalar_mul(smeanR[:], ssumR[:], 1.0 / cfg.NPG)

                # ---- Phase F: MLP
                r1WTs = [constp.tile([HID, 192], f32, name=f"r1w{j}")
                         for j in range(3)]
                for j in range(3):
                    nc.sync.dma_start(r1WTs[j][:], r1WT_d[j * HID:(j + 1) * HID, :])
                r1Bs = [constp.tile([128, 1], f32, name="r1b0"),
                        constp.tile([64, 1], f32, name="r1b1")]
                nc.sync.dma_start(r1Bs[0][:], r1B_d[0:128, :])
                nc.sync.dma_start(r1Bs[1][:], r1B_d[128:192, :])
                r2WTs = [constp.tile([128, cfg.OUT], f32, name="r2w0"),
                         constp.tile([64, cfg.OUT], f32, name="r2w1")]
                nc.sync.dma_start(r2WTs[0][:], r2WT_d[0:128, :])
                nc.sync.dma_start(r2WTs[1][:], r2WT_d[128:192, :])
                r2Bs = constp.tile([cfg.OUT, 1], f32)
                nc.sync.dma_start(r2Bs[:], r2B_d[:])
                chunks = (ssumR, smaxR, smeanR)
                hidT = []
                for mi, (m0, m1) in enumerate(((0, 128), (128, 192))):
                    hps2 = outp.tile([m1 - m0, GPAD], f32, tag="ops", name="hps2")
                    for j in range(3):
                        nc.tensor.matmul(hps2[:], r1WTs[j][:, m0:m1],
                                         chunks[j][:], start=(j == 0), stop=(j == 2))
                    hsb = iop.tile([m1 - m0, GPAD], f32, tag=f"hsb{m0}",
                                   name=f"hsb{m0}")
                    nc.scalar.activation(hsb[:], hps2[:], Act.Lrelu,
                                         bias=r1Bs[mi][:, 0:1], alpha=0.01)
                    hidT.append(hsb)
                yps = outp.tile([cfg.OUT, GPAD], f32, tag="ops")
                nc.tensor.matmul(yps[:], r2WTs[0][:], hidT[0][:],
                                 start=True, stop=False)
                nc.tensor.matmul(yps[:], r2WTs[1][:], hidT[1][:],
                                 start=False, stop=True)
                ysb = iop.tile([cfg.OUT, GPAD], f32, tag="ysb")
                nc.scalar.activation(ysb[:], yps[:], Act.Identity,
                                     bias=r2Bs[:, 0:1])
                nc.sync.dma_start(y_d[:], ysb[:])

    nc.compile()
    return nc


def _wrap_idx(arr):
    w16 = arr.reshape(-1, 16).T
    return np.tile(w16, (8, 1)).copy()


def make_inmaps(cfg, plan, inputs):
    import ml_dtypes
    bf = ml_dtypes.bfloat16
    x = np.asarray(inputs['x'], np.float32)
    kinv = np.array([[1.0 / k] for l in range(cfg.L) for k in range(1, l + 2)],
                    np.float32)
    shared = dict(
        iota=np.tile(np.arange(128, dtype=np.float32)[None, :], (128, 1)),
        ident=np.eye(128, dtype=np.float32),
        embWT=np.ascontiguousarray(np.asarray(inputs['emb_W'], np.float32).T),
        embB=np.asarray(inputs['emb_b'], np.float32)[None, :].copy(),
        convWT=np.ascontiguousarray(
            np.asarray(inputs['conv_W'], np.float32).transpose(0, 2, 1)).astype(bf),
        convB=np.asarray(inputs['conv_b'], np.float32),
        kinv=kinv,
        r1WT=np.ascontiguousarray(np.asarray(inputs['r1_W'], np.float32).T),
        r1B=np.asarray(inputs['r1_b'], np.float32)[:, None].copy(),
        r2WT=np.ascontiguousarray(np.asarray(inputs['r2_W'], np.float32).T),
        r2B=np.asarray(inputs['r2_b'], np.float32)[:, None].copy(),
    )
    in_maps = []
    for c in range(cfg.NCORES):
        m = dict(shared)
        lo_n, hi_n = cfg.core_lo(c), cfg.core_hi(c)
        xs = np.zeros((cfg.NLOC, cfg.IN_DIM), np.float32)
        xs[:hi_n - lo_n] = x[lo_n:hi_n]
        m['xT'] = np.ascontiguousarray(xs.T)
        for k in range(1, cfg.L + 1):
            hp = plan['hops'][k - 1]
            m[f'idx{k}'] = _wrap_idx(hp['idx'][c])
            m[f'dsr{k}'] = np.ascontiguousarray(
                hp['dstrel'][c].reshape(-1, 128).T)
        m['P'] = plan['P'][c]
        in_maps.append(m)
    return in_maps


_CACHE = {}


def kernel(**inputs):
    from concourse.bass_utils import run_bass_kernel_spmd
    cfg = Cfg()
    plan = build_plan(cfg, np.asarray(inputs['k_edge_index']),
                      np.asarray(inputs['batch']))
    if plan['key'] not in _CACHE:
        _CACHE[plan['key']] = build_bass(cfg, plan)
    nc = _CACHE[plan['key']]
    in_maps = make_inmaps(cfg, plan, inputs)
    res = run_bass_kernel_spmd(nc, in_maps, core_ids=list(range(cfg.NCORES)))
    out = np.asarray(res.results[0]['y'], np.float32)
    return np.ascontiguousarray(out[:, :cfg.G].T)



# revision 2
# speedup vs baseline: 1.5278x; 1.5278x over previous
"""DRew-GCN forward on 8 Trainium2 NeuronCores.

Node-partitioned across cores (block-contiguous, graph-aligned). Edges live
with the core owning dst; per hop they are grouped into 128-node dst windows
(uniform tile capacities across cores so one SPMD program serves all 8).
Scatter-add = one-hot matmul into PSUM. The one-hot scatter matrices Q
(Q[e,j] = dinv_k[src_e] * (dstloc[e]==j)) are fully data-dependent host
constants: they are prebuilt on the host per hop (bf16) and streamed from
DRAM, eliminating all on-device degree/coef computation and Q construction.
Rows are fetched by dma_gather from replicated bf16 node tables refreshed per
layer with an AllGather. The GCN projection W is applied after aggregation
(linearity), so gathers move only raw 128-dim rows.
"""
import sys
import numpy as np

if '/opt/trn_rl_repo' not in sys.path:
    sys.path.insert(0, '/opt/trn_rl_repo')


class Cfg:
    def __init__(self, N=50000, G=500, IN_DIM=32, HID=128, OUT=10,
                 L=5, E_K=400000, NCORES=8, CH=32):
        self.N, self.G, self.IN_DIM, self.HID, self.OUT = N, G, IN_DIM, HID, OUT
        self.L, self.E_K, self.NCORES, self.CH = L, E_K, NCORES, CH
        self.NPG = N // G                       # nodes per graph
        assert N % G == 0
        self.GPC = -(-G // NCORES)              # graphs per core (ceil)
        self.BLK = self.GPC * self.NPG          # real nodes per core (last less)
        self.NLOC = 128 * (-(-self.BLK // 128))
        self.W = self.NLOC // 128
        self.TBL = NCORES * self.NLOC
        self.HALF = self.TBL // 2
        assert self.HALF < 32768
        self.GPAD = 128 * (-(-G // 128))
        self.NCONV = L * (L + 1) // 2
        self.SMAX = 64                          # pooling slice tile width
        self.SMAXR = self.GPC                   # real slices per core
        assert self.SMAXR <= self.SMAX

    def core_lo(self, c): return min(self.BLK * c, self.N)
    def core_hi(self, c): return min(self.BLK * (c + 1), self.N)

    def row_of(self, n):
        c = np.minimum(n // self.BLK, self.NCORES - 1)
        return self.NLOC * c + (n - self.BLK * c)


def build_plan(cfg, k_edge_index, batch):
    import ml_dtypes
    bf = ml_dtypes.bfloat16
    NC = cfg.NCORES
    hops = []
    for k in range(1, cfg.L + 1):
        src = np.asarray(k_edge_index[0, (k - 1) * cfg.E_K: k * cfg.E_K], np.int64)
        dst = np.asarray(k_edge_index[1, (k - 1) * cfg.E_K: k * cfg.E_K], np.int64)
        # symmetric-norm degree (in-degree + self-loop), full graph
        deg = np.bincount(dst, minlength=cfg.N).astype(np.float32) + 1.0
        dinv = 1.0 / np.sqrt(deg)
        per_core = []
        for c in range(NC):
            lo_n, hi_n = cfg.core_lo(c), cfg.core_hi(c)
            m = (dst >= lo_n) & (dst < hi_n)
            es, ed = src[m], dst[m]
            selfn = np.arange(lo_n, hi_n, dtype=np.int64)
            es = np.concatenate([es, selfn]); ed = np.concatenate([ed, selfn])
            erow = cfg.row_of(es)
            dloc = ed - lo_n
            half = (erow % 2).astype(np.int64)
            per_core.append((es, erow, dloc, half, dloc // 128))
        T = np.zeros((cfg.W, 2), np.int64)
        for c in range(NC):
            _, _, _, half, w = per_core[c]
            for h in (0, 1):
                cnt = np.bincount(w[half == h], minlength=cfg.W)
                T[:, h] = np.maximum(T[:, h], -(-cnt // 128))
        TL, TH = int(T[:, 0].sum()), int(T[:, 1].sum())
        Ttot = TL + TH
        tile_w = np.concatenate([np.repeat(np.arange(cfg.W), T[:, 0]),
                                 np.repeat(np.arange(cfg.W), T[:, 1])])
        # first/last tile flags per segment and per window (across whole hop)
        seg_first = np.zeros(Ttot, bool); seg_last = np.zeros(Ttot, bool)
        pos = 0
        seg_off = np.zeros((cfg.W, 2), np.int64)
        for h in (0, 1):
            for w in range(cfg.W):
                seg_off[w, h] = pos
                if T[w, h]:
                    seg_first[pos] = True
                    seg_last[pos + T[w, h] - 1] = True
                    pos += int(T[w, h])
        w_first = np.zeros(Ttot, bool); w_last = np.zeros(Ttot, bool)
        for w in range(cfg.W):
            ts = np.nonzero(tile_w == w)[0]
            if len(ts):
                w_first[ts[0]] = True
                w_last[ts[-1]] = True
        idx_all = np.zeros((NC, Ttot * 128), np.int16)
        Q_all = np.zeros((NC, 128, Ttot * 128), bf)
        dok_all = np.zeros((NC, 128, cfg.W), np.float32)
        for c in range(NC):
            es, erow, dloc, half, w = per_core[c]
            for h in (0, 1):
                sel = half == h
                ws, rows, dl, sn = w[sel], erow[sel], dloc[sel], es[sel]
                order = np.argsort(ws, kind='stable')
                ws, rows, dl, sn = ws[order], rows[order], dl[order], sn[order]
                cnts = np.bincount(ws, minlength=cfg.W)
                starts = seg_off[:, h] * 128
                grp0 = np.concatenate([[0], np.cumsum(cnts)[:-1]])
                pos_in = np.arange(len(ws)) - np.repeat(grp0, cnts)
                p = starts[ws] + pos_in
                idx_all[c, p] = (rows >> 1).astype(np.int16)
                # Q[e_lane, tile, dstslot] = dinv_k[src]
                Q_all[c, p % 128, (p // 128) * 128 + (dl - 128 * ws)] = \
                    dinv[sn].astype(bf)
            lo_n, hi_n = cfg.core_lo(c), cfg.core_hi(c)
            nreal = hi_n - lo_n
            dv = np.zeros(cfg.NLOC, np.float32)
            dv[:nreal] = dinv[lo_n:hi_n] / k
            dok_all[c] = dv.reshape(cfg.W, 128).T
        hops.append(dict(T=T, TL=TL, TH=TH, Ttot=Ttot, tile_w=tile_w,
                         seg_first=seg_first, seg_last=seg_last,
                         w_first=w_first, w_last=w_last,
                         idx=idx_all, Q=Q_all, dok=dok_all))
    P = np.zeros((NC, cfg.SMAX, cfg.GPAD), np.float32)
    for c in range(NC):
        lo_n, hi_n = cfg.core_lo(c), cfg.core_hi(c)
        gbase = lo_n // cfg.NPG
        for s in range((hi_n - lo_n) // cfg.NPG):
            P[c, s, gbase + s] = 1.0
    b = np.asarray(batch, np.int64)
    cnt = np.bincount(b, minlength=cfg.G)
    assert (cnt == cfg.NPG).all() and (np.sort(b) == b).all(), \
        "batch must be contiguous-uniform"
    key = tuple(int(h['Ttot']) for h in hops)
    return dict(hops=hops, P=P, key=key)


def build_bass(cfg, plan):
    import concourse.bacc as bacc
    import concourse.mybir as mybir
    from concourse.tile import TileContext
    from concourse.library_config import mlp as mlp_lib

    f32, bf16, i16 = mybir.dt.float32, mybir.dt.bfloat16, mybir.dt.int16
    Alu = mybir.AluOpType
    Act = mybir.ActivationFunctionType
    AX = mybir.AxisListType.X
    NC, L, W, CH = cfg.NCORES, cfg.L, cfg.W, cfg.CH
    HID, GPAD = cfg.HID, cfg.GPAD
    RG = [list(range(NC))]

    nc = bacc.Bacc("TRN2", num_devices=NC)

    xT = nc.dram_tensor("xT", [cfg.IN_DIM, cfg.NLOC], f32, kind="ExternalInput")
    idx_d, q_d, dok_d = [], [], []
    for k in range(1, L + 1):
        hp = plan['hops'][k - 1]
        idx_d.append(nc.dram_tensor(f"idx{k}", [128, hp['Ttot'] * 8], i16,
                                    kind="ExternalInput"))
        q_d.append(nc.dram_tensor(f"q{k}", [128, hp['Ttot'] * 128], bf16,
                                  kind="ExternalInput"))
        dok_d.append(nc.dram_tensor(f"dok{k}", [128, W], f32,
                                    kind="ExternalInput"))
    ident_d = nc.dram_tensor("ident", [128, 128], f32, kind="ExternalInput")
    P_d = nc.dram_tensor("P", [cfg.SMAX, GPAD], f32, kind="ExternalInput")
    embWT_d = nc.dram_tensor("embWT", [cfg.IN_DIM, HID], f32, kind="ExternalInput")
    embB_d = nc.dram_tensor("embB", [1, HID], f32, kind="ExternalInput")
    convWT_d = nc.dram_tensor("convWT", [cfg.NCONV, HID, HID], bf16,
                              kind="ExternalInput")
    convB_d = nc.dram_tensor("convB", [cfg.NCONV, HID], f32, kind="ExternalInput")
    kinv_d = nc.dram_tensor("kinv", [cfg.NCONV, 1], f32, kind="ExternalInput")
    r1WT_d = nc.dram_tensor("r1WT", [3 * HID, 192], f32, kind="ExternalInput")
    r1B_d = nc.dram_tensor("r1B", [192, 1], f32, kind="ExternalInput")
    r2WT_d = nc.dram_tensor("r2WT", [192, cfg.OUT], f32, kind="ExternalInput")
    r2B_d = nc.dram_tensor("r2B", [cfg.OUT, 1], f32, kind="ExternalInput")
    y_d = nc.dram_tensor("y", [cfg.OUT, GPAD], f32, kind="ExternalOutput")

    tables = [nc.dram_tensor(f"tbl{j}", [cfg.TBL, HID], bf16, kind="Internal",
                             addr_space="Shared") for j in range(L)]
    hin = nc.dram_tensor("hin", [cfg.NLOC, HID], bf16, kind="Internal")
    ps_in = [nc.dram_tensor(f"pool_in{i}", [128, GPAD], f32, kind="Internal")
             for i in range(2)]
    ps_out = [nc.dram_tensor(f"pool_out{i}", [128, GPAD], f32, kind="Internal",
                             addr_space="Shared") for i in range(2)]

    hview = hin.rearrange("(w j) f -> j w f", j=128)

    with TileContext(nc) as tc:
        nc.gpsimd.load_library(mlp_lib)
        with tc.tile_pool(name="const", bufs=1) as constp, \
             tc.tile_pool(name="persist", bufs=1) as pers, \
             tc.tile_pool(name="io", bufs=2) as iop, \
             tc.tile_pool(name="msg", bufs=3) as msgp, \
             tc.tile_pool(name="qstream", bufs=3) as qsp, \
             tc.tile_pool(name="qp", bufs=4) as qp, \
             tc.tile_pool(name="agg", bufs=3, space="PSUM") as aggp, \
             tc.tile_pool(name="outp", bufs=2, space="PSUM") as outp, \
             tc.tile_pool(name="smallps", bufs=2, space="PSUM") as smallp:

            ident = constp.tile([128, 128], f32)
            nc.sync.dma_start(ident[:], ident_d[:])
            ones_row = constp.tile([1, 128], f32)
            nc.vector.memset(ones_row[:], 1.0)

            xk = pers.tile([128, W, HID], f32)
            dok = [pers.tile([128, W], f32, tag=f"dok{k}", name=f"dok{k}")
                   for k in range(L)]
            h5T = pers.tile([128, W, 128], f32)
            for k in range(L):
                nc.sync.dma_start(dok[k][:], dok_d[k][:])

            # ---- Phase A: h0 = x @ embW^T + emb_b
            embWT = constp.tile([cfg.IN_DIM, HID], f32)
            nc.sync.dma_start(embWT[:], embWT_d[:])
            embB = constp.tile([1, HID], f32)
            nc.sync.dma_start(embB[:], embB_d[:])
            xTs = pers.tile([cfg.IN_DIM, cfg.NLOC], f32)
            nc.sync.dma_start(xTs[:], xT[:])
            bias_ps = smallp.tile([128, 128], f32, tag="smallt")
            nc.tensor.matmul(bias_ps[:], ones_row[:], embB[:])
            embB_bc = constp.tile([128, 128], f32)
            nc.vector.tensor_copy(embB_bc[:], bias_ps[:])
            h0bf = pers.tile([128, W, HID], bf16, tag="hstage")
            for w in range(W):
                hps = outp.tile([128, HID], f32, tag="ops")
                nc.tensor.matmul(hps[:], xTs[:, w * 128:(w + 1) * 128], embWT[:])
                nc.vector.tensor_tensor(h0bf[:, w, :], hps[:], embB_bc[:], Alu.add)
            nc.sync.dma_start(hview[:, :, :], h0bf[:])
            nc.gpsimd.collective_compute("AllGather", Alu.bypass,
                                         replica_groups=RG,
                                         ins=[hin[:]], outs=[tables[0][:]])

            def gather_stream(k, table, on_chunk):
                hp = plan['hops'][k - 1]
                for (h, lim0, lim1) in ((0, 0, hp['TL']),
                                        (1, hp['TL'], hp['Ttot'])):
                    t0 = lim0
                    while t0 < lim1:
                        n_t = min(CH, lim1 - t0)
                        islab = iop.tile([128, CH * 8], i16, tag="islab")
                        nc.sync.dma_start(islab[:, :n_t * 8],
                                          idx_d[k - 1][:, t0 * 8:(t0 + n_t) * 8])
                        buf = msgp.tile([128, CH, HID], bf16, tag="gbuf")
                        tv = table.rearrange("(r two) f -> r (two f)", two=2)
                        src = tv[:, 0:HID] if h == 0 else tv[:, HID:2 * HID]
                        nc.gpsimd.dma_gather(buf[:, 0:n_t, :], src,
                                             islab[:, :n_t * 8],
                                             n_t * 128, n_t * 128, HID,
                                             elem_step=2 * HID,
                                             single_packet=False)
                        on_chunk(t0, n_t, buf)
                        t0 += n_t

            # ---- Phase D: layers
            for l in range(L):
                nc.vector.memset(xk[:], 0.0)
                for k in range(l + 1, 0, -1):
                    ci = l * (l + 1) // 2 + (k - 1)
                    hp = plan['hops'][k - 1]
                    tbl_j = tables[l - k + 1]
                    wt = iop.tile([128, HID], bf16, tag="wt")
                    nc.sync.dma_start(wt[:], convWT_d[ci, :, :])
                    state = {}

                    def d_chunk(t0, n_t, buf, k=k, hp=hp, wt=wt, state=state):
                        qbuf = qsp.tile([128, CH * 128], bf16, tag="qb")
                        nc.sync.dma_start(qbuf[:, :n_t * 128],
                                          q_d[k - 1][:, t0 * 128:(t0 + n_t) * 128])
                        for i in range(n_t):
                            t = t0 + i
                            w = int(hp['tile_w'][t])
                            if hp['seg_first'][t]:
                                state['agg'] = aggp.tile(
                                    [128, 128], f32, tag="agg", name="aggt")
                            nc.tensor.matmul(state['agg'][:], buf[:, i, :],
                                             qbuf[:, i * 128:(i + 1) * 128],
                                             start=bool(hp['seg_first'][t]),
                                             stop=bool(hp['seg_last'][t]))
                            if hp['seg_last'][t]:
                                at = qp.tile([128, 128], bf16, tag="at")
                                nc.scalar.copy(at[:], state['agg'][:])
                                ops = outp.tile([128, 128], f32, tag="ops")
                                nc.tensor.matmul(ops[:], at[:], wt[:])
                                nc.vector.scalar_tensor_tensor(
                                    xk[:, w, :], ops[:], dok[k - 1][:, w:w + 1],
                                    xk[:, w, :], Alu.mult, Alu.add)
                    gather_stream(k, tbl_j, d_chunk)
                # bias: xk += bcast(sum_ci convB[ci] / k)
                c0 = l * (l + 1) // 2
                bsc = iop.tile([l + 1, HID], f32, tag="bsc")
                nc.sync.dma_start(bsc[:], convB_d[c0:c0 + l + 1, :])
                kv = iop.tile([l + 1, 1], f32, tag="kv")
                nc.sync.dma_start(kv[:], kinv_d[c0:c0 + l + 1, :])
                bscl = iop.tile([l + 1, HID], f32, tag="bscl")
                nc.vector.tensor_scalar_mul(bscl[:], bsc[:], kv[:, 0:1])
                ones_col = iop.tile([l + 1, 1], f32, tag="onescol")
                nc.vector.memset(ones_col[:], 1.0)
                brow_ps = smallp.tile([1, HID], f32, tag="smallt")
                nc.tensor.matmul(brow_ps[:], ones_col[:], bscl[:])
                brow = iop.tile([1, HID], f32, tag="brows")
                nc.vector.tensor_copy(brow[:], brow_ps[:])
                bbc_ps = smallp.tile([128, HID], f32, tag="smallt")
                nc.tensor.matmul(bbc_ps[:], ones_row[:], brow[:])
                bbc = iop.tile([128, HID], f32, tag="bbcs")
                nc.vector.tensor_copy(bbc[:], bbc_ps[:])
                nc.vector.tensor_tensor(xk[:], xk[:],
                                        bbc[:, None, :].broadcast_to(
                                            [128, W, HID]), Alu.add)
                if l < L - 1:
                    hbf = pers.tile([128, W, HID], bf16, tag="hstage")
                    nc.scalar.activation(hbf[:], xk[:], Act.Relu)
                    nc.sync.dma_start(hview[:, :, :], hbf[:])
                    nc.gpsimd.collective_compute(
                        "AllGather", Alu.bypass, replica_groups=RG,
                        ins=[hin[:]], outs=[tables[l + 1][:]])
                else:
                    nc.scalar.activation(xk[:], xk[:], Act.Relu)

            # ---- Phase E: pooling
            for w in range(W):
                tp = outp.tile([128, 128], f32, tag="ops")
                nc.tensor.transpose(tp[:], xk[:, w, :], ident[:])
                nc.vector.tensor_copy(h5T[:, w, :], tp[:])
            h5flat = h5T[:].rearrange("p w j -> p (w j)")
            ssum_l = iop.tile([128, cfg.SMAX], f32, tag="ssum_l")
            smax_l = iop.tile([128, cfg.SMAX], f32, tag="smax_l")
            nc.vector.memset(ssum_l[:], 0.0)
            nc.vector.memset(smax_l[:], 0.0)
            for s in range(cfg.SMAXR):
                sl = h5flat[:, s * cfg.NPG:(s + 1) * cfg.NPG]
                nc.vector.tensor_reduce(ssum_l[:, s:s + 1], sl, axis=AX,
                                        op=Alu.add)
                nc.vector.tensor_reduce(smax_l[:, s:s + 1], sl, axis=AX,
                                        op=Alu.max)
            Ps = constp.tile([cfg.SMAX, GPAD], f32)
            nc.sync.dma_start(Ps[:], P_d[:])
            ssum_t = iop.tile([cfg.SMAX, 128], f32, tag="ssum_t")
            smax_t = iop.tile([cfg.SMAX, 128], f32, tag="smax_t")
            for tsb, tsl in ((ssum_t, ssum_l), (smax_t, smax_l)):
                tps2 = smallp.tile([cfg.SMAX, 128], f32, tag="smallt",
                                   name="tps2")
                nc.tensor.transpose(tps2[:], tsl[:], ident[:])
                nc.vector.tensor_copy(tsb[:], tps2[:])
            for i, tsrc in enumerate((ssum_t, smax_t)):
                pps = outp.tile([128, GPAD], f32, tag="ops")
                nc.tensor.matmul(pps[:], tsrc[:], Ps[:])
                psb = iop.tile([128, GPAD], f32, tag=f"psb{i}")
                nc.vector.tensor_copy(psb[:], pps[:])
                nc.sync.dma_start(ps_in[i][:], psb[:])
            nc.gpsimd.collective_compute("AllReduce", Alu.add, replica_groups=RG,
                                         ins=[ps_in[0][:]], outs=[ps_out[0][:]])
            nc.gpsimd.collective_compute("AllReduce", Alu.max, replica_groups=RG,
                                         ins=[ps_in[1][:]], outs=[ps_out[1][:]])
            ssumR = pers.tile([128, GPAD], f32)
            nc.sync.dma_start(ssumR[:], ps_out[0][:])
            smaxR = pers.tile([128, GPAD], f32)
            nc.sync.dma_start(smaxR[:], ps_out[1][:])
            smeanR = pers.tile([128, GPAD], f32)
            nc.vector.tensor_scalar_mul(smeanR[:], ssumR[:], 1.0 / cfg.NPG)

            # ---- Phase F: MLP
            r1WTs = [constp.tile([HID, 192], f32, name=f"r1w{j}")
                     for j in range(3)]
            for j in range(3):
                nc.sync.dma_start(r1WTs[j][:], r1WT_d[j * HID:(j + 1) * HID, :])
            r1Bs = [constp.tile([128, 1], f32, name="r1b0"),
                    constp.tile([64, 1], f32, name="r1b1")]
            nc.sync.dma_start(r1Bs[0][:], r1B_d[0:128, :])
            nc.sync.dma_start(r1Bs[1][:], r1B_d[128:192, :])
            r2WTs = [constp.tile([128, cfg.OUT], f32, name="r2w0"),
                     constp.tile([64, cfg.OUT], f32, name="r2w1")]
            nc.sync.dma_start(r2WTs[0][:], r2WT_d[0:128, :])
            nc.sync.dma_start(r2WTs[1][:], r2WT_d[128:192, :])
            r2Bs = constp.tile([cfg.OUT, 1], f32)
            nc.sync.dma_start(r2Bs[:], r2B_d[:])
            chunks = (ssumR, smaxR, smeanR)
            hidT = []
            for mi, (m0, m1) in enumerate(((0, 128), (128, 192))):
                hps2 = outp.tile([m1 - m0, GPAD], f32, tag="ops", name="hps2")
                for j in range(3):
                    nc.tensor.matmul(hps2[:], r1WTs[j][:, m0:m1],
                                     chunks[j][:], start=(j == 0), stop=(j == 2))
                hsb = iop.tile([m1 - m0, GPAD], f32, tag=f"hsb{m0}",
                               name=f"hsb{m0}")
                nc.scalar.activation(hsb[:], hps2[:], Act.Lrelu,
                                     bias=r1Bs[mi][:, 0:1], alpha=0.01)
                hidT.append(hsb)
            yps = outp.tile([cfg.OUT, GPAD], f32, tag="ops")
            nc.tensor.matmul(yps[:], r2WTs[0][:], hidT[0][:],
                             start=True, stop=False)
            nc.tensor.matmul(yps[:], r2WTs[1][:], hidT[1][:],
                             start=False, stop=True)
            ysb = iop.tile([cfg.OUT, GPAD], f32, tag="ysb")
            nc.scalar.activation(ysb[:], yps[:], Act.Identity,
                                 bias=r2Bs[:, 0:1])
            nc.sync.dma_start(y_d[:], ysb[:])

    nc.compile()
    return nc


def _wrap_idx(arr):
    w16 = arr.reshape(-1, 16).T
    return np.tile(w16, (8, 1)).copy()


def make_inmaps(cfg, plan, inputs):
    import ml_dtypes
    bf = ml_dtypes.bfloat16
    x = np.asarray(inputs['x'], np.float32)
    kinv = np.array([[1.0 / k] for l in range(cfg.L) for k in range(1, l + 2)],
                    np.float32)
    shared = dict(
        ident=np.eye(128, dtype=np.float32),
        embWT=np.ascontiguousarray(np.asarray(inputs['emb_W'], np.float32).T),
        embB=np.asarray(inputs['emb_b'], np.float32)[None, :].copy(),
        convWT=np.ascontiguousarray(
            np.asarray(inputs['conv_W'], np.float32).transpose(0, 2, 1)).astype(bf),
        convB=np.asarray(inputs['conv_b'], np.float32),
        kinv=kinv,
        r1WT=np.ascontiguousarray(np.asarray(inputs['r1_W'], np.float32).T),
        r1B=np.asarray(inputs['r1_b'], np.float32)[:, None].copy(),
        r2WT=np.ascontiguousarray(np.asarray(inputs['r2_W'], np.float32).T),
        r2B=np.asarray(inputs['r2_b'], np.float32)[:, None].copy(),
    )
    in_maps = []
    for c in range(cfg.NCORES):
        m = dict(shared)
        lo_n, hi_n = cfg.core_lo(c), cfg.core_hi(c)
        xs = np.zeros((cfg.NLOC, cfg.IN_DIM), np.float32)
        xs[:hi_n - lo_n] = x[lo_n:hi_n]
        m['xT'] = np.ascontiguousarray(xs.T)
        for k in range(1, cfg.L + 1):
            hp = plan['hops'][k - 1]
            m[f'idx{k}'] = _wrap_idx(hp['idx'][c])
            m[f'q{k}'] = hp['Q'][c]
            m[f'dok{k}'] = hp['dok'][c]
        m['P'] = plan['P'][c]
        in_maps.append(m)
    return in_maps


_CACHE = {}


def kernel(**inputs):
    from concourse.bass_utils import run_bass_kernel_spmd
    cfg = Cfg()
    plan = build_plan(cfg, np.asarray(inputs['k_edge_index']),
                      np.asarray(inputs['batch']))
    if plan['key'] not in _CACHE:
        _CACHE[plan['key']] = build_bass(cfg, plan)
    nc = _CACHE[plan['key']]
    in_maps = make_inmaps(cfg, plan, inputs)
    res = run_bass_kernel_spmd(nc, in_maps, core_ids=list(range(cfg.NCORES)))
    out = np.asarray(res.results[0]['y'], np.float32)
    return np.ascontiguousarray(out[:, :cfg.G].T)


# revision 3
# speedup vs baseline: 1.5279x; 1.0001x over previous
"""DRew-GCN forward on 8 Trainium2 NeuronCores.

Node-partitioned across cores (block-contiguous, graph-aligned). Edges live
with the core owning dst; per hop they are grouped into 128-node dst windows
(uniform tile capacities across cores so one SPMD program serves all 8).
Scatter-add = one-hot matmul into PSUM. The one-hot scatter matrices Q
(Q[e,j] = dinv_k[src_e] * (dstloc[e]==j)) are fully data-dependent host
constants: prebuilt per hop (bf16) and streamed from DRAM, so no on-device
degree/coef computation or Q construction. Self-loops are excluded from the
edge stream; the self term is a per-window dense matmul against a host-built
scaled diagonal. Rows are fetched by dma_gather from replicated bf16 node
tables refreshed per layer with an AllGather. The GCN projection W is applied
after aggregation (linearity), so gathers move only raw 128-dim rows.
Pooling + MLP are core-local (batch is graph-aligned); the host concatenates
the per-core outputs, so no AllReduce is needed.
"""
import sys
import numpy as np

if '/opt/trn_rl_repo' not in sys.path:
    sys.path.insert(0, '/opt/trn_rl_repo')


class Cfg:
    def __init__(self, N=50000, G=500, IN_DIM=32, HID=128, OUT=10,
                 L=5, E_K=400000, NCORES=8, CH=32):
        self.N, self.G, self.IN_DIM, self.HID, self.OUT = N, G, IN_DIM, HID, OUT
        self.L, self.E_K, self.NCORES, self.CH = L, E_K, NCORES, CH
        self.NPG = N // G                       # nodes per graph
        assert N % G == 0
        self.GPC = -(-G // NCORES)              # graphs per core (ceil)
        self.BLK = self.GPC * self.NPG          # real nodes per core (last less)
        self.NLOC = 128 * (-(-self.BLK // 128))
        self.W = self.NLOC // 128
        self.TBL = NCORES * self.NLOC
        self.HALF = self.TBL // 2
        assert self.HALF < 32768
        self.NCONV = L * (L + 1) // 2
        self.SMAX = 64                          # pooling slice tile width
        self.SMAXR = self.GPC                   # real slices per core
        assert self.SMAXR <= self.SMAX

    def core_lo(self, c): return min(self.BLK * c, self.N)
    def core_hi(self, c): return min(self.BLK * (c + 1), self.N)

    def row_of(self, n):
        c = np.minimum(n // self.BLK, self.NCORES - 1)
        return self.NLOC * c + (n - self.BLK * c)


def build_plan(cfg, k_edge_index, batch):
    import ml_dtypes
    bf = ml_dtypes.bfloat16
    NC = cfg.NCORES
    hops = []
    for k in range(1, cfg.L + 1):
        src = np.asarray(k_edge_index[0, (k - 1) * cfg.E_K: k * cfg.E_K], np.int64)
        dst = np.asarray(k_edge_index[1, (k - 1) * cfg.E_K: k * cfg.E_K], np.int64)
        # symmetric-norm degree (in-degree + self-loop), full graph
        deg = np.bincount(dst, minlength=cfg.N).astype(np.float32) + 1.0
        dinv = 1.0 / np.sqrt(deg)
        per_core = []
        for c in range(NC):
            lo_n, hi_n = cfg.core_lo(c), cfg.core_hi(c)
            m = (dst >= lo_n) & (dst < hi_n)
            es, ed = src[m], dst[m]
            erow = cfg.row_of(es)
            dloc = ed - lo_n
            half = (erow % 2).astype(np.int64)
            per_core.append((es, erow, dloc, half, dloc // 128))
        T = np.zeros((cfg.W, 2), np.int64)
        for c in range(NC):
            _, _, _, half, w = per_core[c]
            for h in (0, 1):
                cnt = np.bincount(w[half == h], minlength=cfg.W)
                T[:, h] = np.maximum(T[:, h], -(-cnt // 128))
        TL, TH = int(T[:, 0].sum()), int(T[:, 1].sum())
        Ttot = TL + TH
        tile_w = np.concatenate([np.repeat(np.arange(cfg.W), T[:, 0]),
                                 np.repeat(np.arange(cfg.W), T[:, 1])])
        seg_first = np.zeros(Ttot, bool); seg_last = np.zeros(Ttot, bool)
        pos = 0
        seg_off = np.zeros((cfg.W, 2), np.int64)
        for h in (0, 1):
            for w in range(cfg.W):
                seg_off[w, h] = pos
                if T[w, h]:
                    seg_first[pos] = True
                    seg_last[pos + T[w, h] - 1] = True
                    pos += int(T[w, h])
        idx_all = np.zeros((NC, Ttot * 128), np.int16)
        Q_all = np.zeros((NC, 128, Ttot * 128), bf)
        diagq_all = np.zeros((NC, 128, cfg.W * 128), bf)
        dok_all = np.zeros((NC, 128, cfg.W), np.float32)
        for c in range(NC):
            es, erow, dloc, half, w = per_core[c]
            for h in (0, 1):
                sel = half == h
                ws, rows, dl, sn = w[sel], erow[sel], dloc[sel], es[sel]
                order = np.argsort(ws, kind='stable')
                ws, rows, dl, sn = ws[order], rows[order], dl[order], sn[order]
                cnts = np.bincount(ws, minlength=cfg.W)
                starts = seg_off[:, h] * 128
                grp0 = np.concatenate([[0], np.cumsum(cnts)[:-1]])
                pos_in = np.arange(len(ws)) - np.repeat(grp0, cnts)
                p = starts[ws] + pos_in
                idx_all[c, p] = (rows >> 1).astype(np.int16)
                Q_all[c, p % 128, (p // 128) * 128 + (dl - 128 * ws)] = \
                    dinv[sn].astype(bf)
            lo_n, hi_n = cfg.core_lo(c), cfg.core_hi(c)
            nreal = hi_n - lo_n
            dv = np.zeros(cfg.NLOC, np.float32)
            dv[:nreal] = dinv[lo_n:hi_n] / k
            dok_all[c] = dv.reshape(cfg.W, 128).T
            dself = np.zeros(cfg.NLOC, np.float32)
            dself[:nreal] = dinv[lo_n:hi_n]
            lanes = np.arange(cfg.NLOC)
            diagq_all[c, lanes % 128, (lanes // 128) * 128 + lanes % 128] = \
                dself.astype(bf)
        hops.append(dict(T=T, TL=TL, TH=TH, Ttot=Ttot, tile_w=tile_w,
                         seg_first=seg_first, seg_last=seg_last,
                         idx=idx_all, Q=Q_all, diagq=diagq_all, dok=dok_all))
    b = np.asarray(batch, np.int64)
    cnt = np.bincount(b, minlength=cfg.G)
    assert (cnt == cfg.NPG).all() and (np.sort(b) == b).all(), \
        "batch must be contiguous-uniform"
    key = tuple(int(h['Ttot']) for h in hops)
    return dict(hops=hops, key=key)


def build_bass(cfg, plan):
    import concourse.bacc as bacc
    import concourse.mybir as mybir
    from concourse.tile import TileContext
    from concourse.library_config import mlp as mlp_lib

    f32, bf16, i16 = mybir.dt.float32, mybir.dt.bfloat16, mybir.dt.int16
    Alu = mybir.AluOpType
    Act = mybir.ActivationFunctionType
    AX = mybir.AxisListType.X
    NC, L, W, CH = cfg.NCORES, cfg.L, cfg.W, cfg.CH
    HID = cfg.HID
    SM = cfg.SMAX
    RG = [list(range(NC))]

    nc = bacc.Bacc("TRN2", num_devices=NC)

    xT = nc.dram_tensor("xT", [cfg.IN_DIM, cfg.NLOC], f32, kind="ExternalInput")
    idx_d, q_d, dq_d, dok_d = [], [], [], []
    for k in range(1, L + 1):
        hp = plan['hops'][k - 1]
        idx_d.append(nc.dram_tensor(f"idx{k}", [128, hp['Ttot'] * 8], i16,
                                    kind="ExternalInput"))
        q_d.append(nc.dram_tensor(f"q{k}", [128, hp['Ttot'] * 128], bf16,
                                  kind="ExternalInput"))
        dq_d.append(nc.dram_tensor(f"dq{k}", [128, W * 128], bf16,
                                   kind="ExternalInput"))
        dok_d.append(nc.dram_tensor(f"dok{k}", [128, W], f32,
                                    kind="ExternalInput"))
    ident_d = nc.dram_tensor("ident", [128, 128], f32, kind="ExternalInput")
    embWT_d = nc.dram_tensor("embWT", [cfg.IN_DIM, HID], f32, kind="ExternalInput")
    embB_d = nc.dram_tensor("embB", [1, HID], f32, kind="ExternalInput")
    convWT_d = nc.dram_tensor("convWT", [cfg.NCONV, HID, HID], bf16,
                              kind="ExternalInput")
    convB_d = nc.dram_tensor("convB", [cfg.NCONV, HID], f32, kind="ExternalInput")
    kinv_d = nc.dram_tensor("kinv", [cfg.NCONV, 1], f32, kind="ExternalInput")
    r1WT_d = nc.dram_tensor("r1WT", [3 * HID, 192], f32, kind="ExternalInput")
    r1B_d = nc.dram_tensor("r1B", [192, 1], f32, kind="ExternalInput")
    r2WT_d = nc.dram_tensor("r2WT", [192, cfg.OUT], f32, kind="ExternalInput")
    r2B_d = nc.dram_tensor("r2B", [cfg.OUT, 1], f32, kind="ExternalInput")
    y_d = nc.dram_tensor("y", [cfg.OUT, SM], f32, kind="ExternalOutput")

    tables = [nc.dram_tensor(f"tbl{j}", [cfg.TBL, HID], bf16, kind="Internal",
                             addr_space="Shared") for j in range(L)]
    hin = nc.dram_tensor("hin", [cfg.NLOC, HID], bf16, kind="Internal")
    ownblk = [nc.dram_tensor(f"own{j}", [128, W * HID], bf16, kind="Internal")
              for j in range(L)]

    hview = hin.rearrange("(w j) f -> j w f", j=128)

    with TileContext(nc) as tc:
        nc.gpsimd.load_library(mlp_lib)
        with tc.tile_pool(name="const", bufs=1) as constp, \
             tc.tile_pool(name="persist", bufs=1) as pers, \
             tc.tile_pool(name="io", bufs=2) as iop, \
             tc.tile_pool(name="own", bufs=1) as ownp, \
             tc.tile_pool(name="msg", bufs=3) as msgp, \
             tc.tile_pool(name="qstream", bufs=3) as qsp, \
             tc.tile_pool(name="qp", bufs=4) as qp, \
             tc.tile_pool(name="agg", bufs=3, space="PSUM") as aggp, \
             tc.tile_pool(name="outp", bufs=2, space="PSUM") as outp, \
             tc.tile_pool(name="smallps", bufs=2, space="PSUM") as smallp:

            ident = constp.tile([128, 128], f32)
            nc.sync.dma_start(ident[:], ident_d[:])
            ones_row = constp.tile([1, 128], f32)
            nc.vector.memset(ones_row[:], 1.0)

            xk = pers.tile([128, W, HID], f32)
            dok = [pers.tile([128, W], f32, tag=f"dok{k}", name=f"dok{k}")
                   for k in range(L)]
            h5T = pers.tile([128, W, 128], f32)
            for k in range(L):
                nc.sync.dma_start(dok[k][:], dok_d[k][:])

            # ---- Phase A: h0 = x @ embW^T + emb_b
            embWT = constp.tile([cfg.IN_DIM, HID], f32)
            nc.sync.dma_start(embWT[:], embWT_d[:])
            embB = constp.tile([1, HID], f32)
            nc.sync.dma_start(embB[:], embB_d[:])
            xTs = pers.tile([cfg.IN_DIM, cfg.NLOC], f32)
            nc.sync.dma_start(xTs[:], xT[:])
            bias_ps = smallp.tile([128, 128], f32, tag="smallt")
            nc.tensor.matmul(bias_ps[:], ones_row[:], embB[:])
            embB_bc = constp.tile([128, 128], f32)
            nc.vector.tensor_copy(embB_bc[:], bias_ps[:])
            h0bf = pers.tile([128, W, HID], bf16, tag="hstage")
            for w in range(W):
                hps = outp.tile([128, HID], f32, tag="ops")
                nc.tensor.matmul(hps[:], xTs[:, w * 128:(w + 1) * 128], embWT[:])
                nc.vector.tensor_tensor(h0bf[:, w, :], hps[:], embB_bc[:], Alu.add)
            nc.sync.dma_start(hview[:, :, :], h0bf[:])
            nc.sync.dma_start(ownblk[0][:],
                              h0bf[:].rearrange("p w f -> p (w f)"))
            nc.gpsimd.collective_compute("AllGather", Alu.bypass,
                                         replica_groups=RG,
                                         ins=[hin[:]], outs=[tables[0][:]])

            def gather_stream(k, table, on_chunk):
                hp = plan['hops'][k - 1]
                for (h, lim0, lim1) in ((0, 0, hp['TL']),
                                        (1, hp['TL'], hp['Ttot'])):
                    t0 = lim0
                    while t0 < lim1:
                        n_t = min(CH, lim1 - t0)
                        islab = iop.tile([128, CH * 8], i16, tag="islab")
                        nc.sync.dma_start(islab[:, :n_t * 8],
                                          idx_d[k - 1][:, t0 * 8:(t0 + n_t) * 8])
                        buf = msgp.tile([128, CH, HID], bf16, tag="gbuf")
                        tv = table.rearrange("(r two) f -> r (two f)", two=2)
                        src = tv[:, 0:HID] if h == 0 else tv[:, HID:2 * HID]
                        nc.gpsimd.dma_gather(buf[:, 0:n_t, :], src,
                                             islab[:, :n_t * 8],
                                             n_t * 128, n_t * 128, HID,
                                             elem_step=2 * HID,
                                             single_packet=False)
                        on_chunk(t0, n_t, buf)
                        t0 += n_t

            # ---- Phase D: layers
            for l in range(L):
                nc.vector.memset(xk[:], 0.0)
                for k in range(l + 1, 0, -1):
                    ci = l * (l + 1) // 2 + (k - 1)
                    hp = plan['hops'][k - 1]
                    j = l - k + 1
                    tbl_j = tables[j]
                    wt = iop.tile([128, HID], bf16, tag="wt")
                    nc.sync.dma_start(wt[:], convWT_d[ci, :, :])
                    # self-loop term: per-window dense matmul vs scaled diag
                    ownb = ownp.tile([128, W, HID], bf16, tag="ownb")
                    nc.sync.dma_start(ownb[:],
                                      ownblk[j].rearrange("p (w f) -> p w f",
                                                          f=HID))
                    dq = ownp.tile([128, W * 128], bf16, tag="dq")
                    nc.sync.dma_start(dq[:], dq_d[k - 1][:])
                    for w in range(W):
                        sagg = aggp.tile([128, 128], f32, tag="agg",
                                         name="saggt")
                        nc.tensor.matmul(sagg[:], ownb[:, w, :],
                                         dq[:, w * 128:(w + 1) * 128],
                                         start=True, stop=True)
                        at2 = qp.tile([128, 128], bf16, tag="at")
                        nc.scalar.copy(at2[:], sagg[:])
                        ops2 = outp.tile([128, 128], f32, tag="ops")
                        nc.tensor.matmul(ops2[:], at2[:], wt[:])
                        nc.vector.scalar_tensor_tensor(
                            xk[:, w, :], ops2[:], dok[k - 1][:, w:w + 1],
                            xk[:, w, :], Alu.mult, Alu.add)
                    state = {}

                    def d_chunk(t0, n_t, buf, k=k, hp=hp, wt=wt, state=state):
                        qbuf = qsp.tile([128, CH * 128], bf16, tag="qb")
                        nc.sync.dma_start(qbuf[:, :n_t * 128],
                                          q_d[k - 1][:, t0 * 128:(t0 + n_t) * 128])
                        for i in range(n_t):
                            t = t0 + i
                            w = int(hp['tile_w'][t])
                            if hp['seg_first'][t]:
                                state['agg'] = aggp.tile(
                                    [128, 128], f32, tag="agg", name="aggt")
                            nc.tensor.matmul(state['agg'][:], buf[:, i, :],
                                             qbuf[:, i * 128:(i + 1) * 128],
                                             start=bool(hp['seg_first'][t]),
                                             stop=bool(hp['seg_last'][t]))
                            if hp['seg_last'][t]:
                                at = qp.tile([128, 128], bf16, tag="at")
                                nc.scalar.copy(at[:], state['agg'][:])
                                ops = outp.tile([128, 128], f32, tag="ops")
                                nc.tensor.matmul(ops[:], at[:], wt[:])
                                nc.vector.scalar_tensor_tensor(
                                    xk[:, w, :], ops[:], dok[k - 1][:, w:w + 1],
                                    xk[:, w, :], Alu.mult, Alu.add)
                    gather_stream(k, tbl_j, d_chunk)
                # bias: xk += bcast(sum_ci convB[ci] / k)
                c0 = l * (l + 1) // 2
                bsc = iop.tile([l + 1, HID], f32, tag="bsc")
                nc.sync.dma_start(bsc[:], convB_d[c0:c0 + l + 1, :])
                kv = iop.tile([l + 1, 1], f32, tag="kv")
                nc.sync.dma_start(kv[:], kinv_d[c0:c0 + l + 1, :])
                bscl = iop.tile([l + 1, HID], f32, tag="bscl")
                nc.vector.tensor_scalar_mul(bscl[:], bsc[:], kv[:, 0:1])
                ones_col = iop.tile([l + 1, 1], f32, tag="onescol")
                nc.vector.memset(ones_col[:], 1.0)
                brow_ps = smallp.tile([1, HID], f32, tag="smallt")
                nc.tensor.matmul(brow_ps[:], ones_col[:], bscl[:])
                brow = iop.tile([1, HID], f32, tag="brows")
                nc.vector.tensor_copy(brow[:], brow_ps[:])
                bbc_ps = smallp.tile([128, HID], f32, tag="smallt")
                nc.tensor.matmul(bbc_ps[:], ones_row[:], brow[:])
                bbc = iop.tile([128, HID], f32, tag="bbcs")
                nc.vector.tensor_copy(bbc[:], bbc_ps[:])
                nc.vector.tensor_tensor(xk[:], xk[:],
                                        bbc[:, None, :].broadcast_to(
                                            [128, W, HID]), Alu.add)
                if l < L - 1:
                    hbf = pers.tile([128, W, HID], bf16, tag="hstage")
                    nc.scalar.activation(hbf[:], xk[:], Act.Relu)
                    nc.sync.dma_start(hview[:, :, :], hbf[:])
                    nc.sync.dma_start(ownblk[l + 1][:],
                                      hbf[:].rearrange("p w f -> p (w f)"))
                    nc.gpsimd.collective_compute(
                        "AllGather", Alu.bypass, replica_groups=RG,
                        ins=[hin[:]], outs=[tables[l + 1][:]])
                else:
                    nc.scalar.activation(xk[:], xk[:], Act.Relu)

            # ---- Phase E: core-local pooling (batch is graph-aligned)
            for w in range(W):
                tp = outp.tile([128, 128], f32, tag="ops")
                nc.tensor.transpose(tp[:], xk[:, w, :], ident[:])
                nc.vector.tensor_copy(h5T[:, w, :], tp[:])
            h5flat = h5T[:].rearrange("p w j -> p (w j)")
            ssum_l = iop.tile([128, SM], f32, tag="ssum_l")
            smax_l = iop.tile([128, SM], f32, tag="smax_l")
            nc.vector.memset(ssum_l[:], 0.0)
            nc.vector.memset(smax_l[:], 0.0)
            for s in range(cfg.SMAXR):
                sl = h5flat[:, s * cfg.NPG:(s + 1) * cfg.NPG]
                nc.vector.tensor_reduce(ssum_l[:, s:s + 1], sl, axis=AX,
                                        op=Alu.add)
                nc.vector.tensor_reduce(smax_l[:, s:s + 1], sl, axis=AX,
                                        op=Alu.max)
            smean_l = iop.tile([128, SM], f32, tag="smean_l")
            nc.vector.tensor_scalar_mul(smean_l[:], ssum_l[:], 1.0 / cfg.NPG)

            # ---- Phase F: core-local MLP on [*, SMAX] graphs
            r1WTs = [constp.tile([HID, 192], f32, name=f"r1w{j}")
                     for j in range(3)]
            for j in range(3):
                nc.sync.dma_start(r1WTs[j][:], r1WT_d[j * HID:(j + 1) * HID, :])
            r1Bs = [constp.tile([128, 1], f32, name="r1b0"),
                    constp.tile([64, 1], f32, name="r1b1")]
            nc.sync.dma_start(r1Bs[0][:], r1B_d[0:128, :])
            nc.sync.dma_start(r1Bs[1][:], r1B_d[128:192, :])
            r2WTs = [constp.tile([128, cfg.OUT], f32, name="r2w0"),
                     constp.tile([64, cfg.OUT], f32, name="r2w1")]
            nc.sync.dma_start(r2WTs[0][:], r2WT_d[0:128, :])
            nc.sync.dma_start(r2WTs[1][:], r2WT_d[128:192, :])
            r2Bs = constp.tile([cfg.OUT, 1], f32)
            nc.sync.dma_start(r2Bs[:], r2B_d[:])
            chunks = (ssum_l, smax_l, smean_l)
            hidT = []
            for mi, (m0, m1) in enumerate(((0, 128), (128, 192))):
                hps2 = outp.tile([m1 - m0, SM], f32, tag="ops", name="hps2")
                for j in range(3):
                    nc.tensor.matmul(hps2[:], r1WTs[j][:, m0:m1],
                                     chunks[j][:], start=(j == 0), stop=(j == 2))
                hsb = iop.tile([m1 - m0, SM], f32, tag=f"hsb{m0}",
                               name=f"hsb{m0}")
                nc.scalar.activation(hsb[:], hps2[:], Act.Lrelu,
                                     bias=r1Bs[mi][:, 0:1], alpha=0.01)
                hidT.append(hsb)
            yps = outp.tile([cfg.OUT, SM], f32, tag="ops")
            nc.tensor.matmul(yps[:], r2WTs[0][:], hidT[0][:],
                             start=True, stop=False)
            nc.tensor.matmul(yps[:], r2WTs[1][:], hidT[1][:],
                             start=False, stop=True)
            ysb = iop.tile([cfg.OUT, SM], f32, tag="ysb")
            nc.scalar.activation(ysb[:], yps[:], Act.Identity,
                                 bias=r2Bs[:, 0:1])
            nc.sync.dma_start(y_d[:], ysb[:])

    nc.compile()
    return nc


def _wrap_idx(arr):
    w16 = arr.reshape(-1, 16).T
    return np.tile(w16, (8, 1)).copy()


def make_inmaps(cfg, plan, inputs):
    import ml_dtypes
    bf = ml_dtypes.bfloat16
    x = np.asarray(inputs['x'], np.float32)
    kinv = np.array([[1.0 / k] for l in range(cfg.L) for k in range(1, l + 2)],
                    np.float32)
    shared = dict(
        ident=np.eye(128, dtype=np.float32),
        embWT=np.ascontiguousarray(np.asarray(inputs['emb_W'], np.float32).T),
        embB=np.asarray(inputs['emb_b'], np.float32)[None, :].copy(),
        convWT=np.ascontiguousarray(
            np.asarray(inputs['conv_W'], np.float32).transpose(0, 2, 1)).astype(bf),
        convB=np.asarray(inputs['conv_b'], np.float32),
        kinv=kinv,
        r1WT=np.ascontiguousarray(np.asarray(inputs['r1_W'], np.float32).T),
        r1B=np.asarray(inputs['r1_b'], np.float32)[:, None].copy(),
        r2WT=np.ascontiguousarray(np.asarray(inputs['r2_W'], np.float32).T),
        r2B=np.asarray(inputs['r2_b'], np.float32)[:, None].copy(),
    )
    in_maps = []
    for c in range(cfg.NCORES):
        m = dict(shared)
        lo_n, hi_n = cfg.core_lo(c), cfg.core_hi(c)
        xs = np.zeros((cfg.NLOC, cfg.IN_DIM), np.float32)
        xs[:hi_n - lo_n] = x[lo_n:hi_n]
        m['xT'] = np.ascontiguousarray(xs.T)
        for k in range(1, cfg.L + 1):
            hp = plan['hops'][k - 1]
            m[f'idx{k}'] = _wrap_idx(hp['idx'][c])
            m[f'q{k}'] = hp['Q'][c]
            m[f'dq{k}'] = hp['diagq'][c]
            m[f'dok{k}'] = hp['dok'][c]
        in_maps.append(m)
    return in_maps


_CACHE = {}


def kernel(**inputs):
    from concourse.bass_utils import run_bass_kernel_spmd
    cfg = Cfg()
    plan = build_plan(cfg, np.asarray(inputs['k_edge_index']),
                      np.asarray(inputs['batch']))
    if plan['key'] not in _CACHE:
        _CACHE[plan['key']] = build_bass(cfg, plan)
    nc = _CACHE[plan['key']]
    in_maps = make_inmaps(cfg, plan, inputs)
    res = run_bass_kernel_spmd(nc, in_maps, core_ids=list(range(cfg.NCORES)))
    parts = []
    for c in range(cfg.NCORES):
        ng = (cfg.core_hi(c) - cfg.core_lo(c)) // cfg.NPG
        yc = np.asarray(res.results[c]['y'], np.float32)
        parts.append(yc[:, :ng].T)
    return np.ascontiguousarray(np.concatenate(parts, axis=0))


# revision 4
# speedup vs baseline: 1.8130x; 1.1866x over previous
"""DRew-GCN forward on 8 Trainium2 NeuronCores.

Node-partitioned across cores (block-contiguous, graph-aligned). Edges live
with the core owning dst; per hop they are grouped into 128-node dst windows
(uniform tile capacities across cores so one SPMD program serves all 8).
Scatter-add = one-hot matmul into PSUM. The one-hot scatter matrices Q
(Q[e,j] = dinv_k[src_e] * (dstloc[e]==j)) are fully data-dependent host
constants: prebuilt per hop (bf16) and streamed from DRAM, so no on-device
degree/coef computation or Q construction. Self-loops are excluded from the
edge stream; the self term is a per-window dense matmul against a host-built
scaled diagonal. Rows are fetched by dma_gather from replicated bf16 node
tables refreshed per layer with an AllGather. The GCN projection W is applied
after aggregation (linearity), so gathers move only raw 128-dim rows.
Pooling + MLP are core-local (batch is graph-aligned); the host concatenates
the per-core outputs, so no AllReduce is needed.
"""
import sys
import numpy as np

if '/opt/trn_rl_repo' not in sys.path:
    sys.path.insert(0, '/opt/trn_rl_repo')


class Cfg:
    def __init__(self, N=50000, G=500, IN_DIM=32, HID=128, OUT=10,
                 L=5, E_K=400000, NCORES=8, CH=32):
        self.N, self.G, self.IN_DIM, self.HID, self.OUT = N, G, IN_DIM, HID, OUT
        self.L, self.E_K, self.NCORES, self.CH = L, E_K, NCORES, CH
        self.NPG = N // G                       # nodes per graph
        assert N % G == 0
        self.GPC = -(-G // NCORES)              # graphs per core (ceil)
        self.BLK = self.GPC * self.NPG          # real nodes per core (last less)
        self.NLOC = 128 * (-(-self.BLK // 128))
        self.W = self.NLOC // 128
        self.TBL = NCORES * self.NLOC
        self.HALF = self.TBL // 2
        assert self.HALF < 32768
        self.NCONV = L * (L + 1) // 2
        self.SMAX = 64                          # pooling slice tile width
        self.SMAXR = self.GPC                   # real slices per core
        assert self.SMAXR <= self.SMAX

    def core_lo(self, c): return min(self.BLK * c, self.N)
    def core_hi(self, c): return min(self.BLK * (c + 1), self.N)

    def row_of(self, n):
        c = np.minimum(n // self.BLK, self.NCORES - 1)
        return self.NLOC * c + (n - self.BLK * c)


def build_plan(cfg, k_edge_index, batch):
    import ml_dtypes
    bf = ml_dtypes.bfloat16
    NC = cfg.NCORES
    hops = []
    for k in range(1, cfg.L + 1):
        src = np.asarray(k_edge_index[0, (k - 1) * cfg.E_K: k * cfg.E_K], np.int64)
        dst = np.asarray(k_edge_index[1, (k - 1) * cfg.E_K: k * cfg.E_K], np.int64)
        # symmetric-norm degree (in-degree + self-loop), full graph
        deg = np.bincount(dst, minlength=cfg.N).astype(np.float32) + 1.0
        dinv = 1.0 / np.sqrt(deg)
        per_core = []
        for c in range(NC):
            lo_n, hi_n = cfg.core_lo(c), cfg.core_hi(c)
            m = (dst >= lo_n) & (dst < hi_n)
            es, ed = src[m], dst[m]
            erow = cfg.row_of(es)
            dloc = ed - lo_n
            half = (erow % 2).astype(np.int64)
            per_core.append((es, erow, dloc, half, dloc // 128))
        T = np.zeros((cfg.W, 2), np.int64)
        for c in range(NC):
            _, _, _, half, w = per_core[c]
            for h in (0, 1):
                cnt = np.bincount(w[half == h], minlength=cfg.W)
                T[:, h] = np.maximum(T[:, h], -(-cnt // 128))
        TL, TH = int(T[:, 0].sum()), int(T[:, 1].sum())
        Ttot = TL + TH
        tile_w = np.concatenate([np.repeat(np.arange(cfg.W), T[:, 0]),
                                 np.repeat(np.arange(cfg.W), T[:, 1])])
        seg_first = np.zeros(Ttot, bool); seg_last = np.zeros(Ttot, bool)
        pos = 0
        seg_off = np.zeros((cfg.W, 2), np.int64)
        for h in (0, 1):
            for w in range(cfg.W):
                seg_off[w, h] = pos
                if T[w, h]:
                    seg_first[pos] = True
                    seg_last[pos + T[w, h] - 1] = True
                    pos += int(T[w, h])
        idx_all = np.zeros((NC, Ttot * 128), np.int16)
        Q_all = np.zeros((NC, 128, Ttot * 128), bf)
        diagq_all = np.zeros((NC, 128, cfg.W * 128), bf)
        dok_all = np.zeros((NC, 128, cfg.W), np.float32)
        for c in range(NC):
            es, erow, dloc, half, w = per_core[c]
            for h in (0, 1):
                sel = half == h
                ws, rows, dl, sn = w[sel], erow[sel], dloc[sel], es[sel]
                order = np.argsort(ws, kind='stable')
                ws, rows, dl, sn = ws[order], rows[order], dl[order], sn[order]
                cnts = np.bincount(ws, minlength=cfg.W)
                starts = seg_off[:, h] * 128
                grp0 = np.concatenate([[0], np.cumsum(cnts)[:-1]])
                pos_in = np.arange(len(ws)) - np.repeat(grp0, cnts)
                p = starts[ws] + pos_in
                idx_all[c, p] = (rows >> 1).astype(np.int16)
                Q_all[c, p % 128, (p // 128) * 128 + (dl - 128 * ws)] = \
                    dinv[sn].astype(bf)
            lo_n, hi_n = cfg.core_lo(c), cfg.core_hi(c)
            nreal = hi_n - lo_n
            dv = np.zeros(cfg.NLOC, np.float32)
            dv[:nreal] = dinv[lo_n:hi_n] / k
            dok_all[c] = dv.reshape(cfg.W, 128).T
            dself = np.zeros(cfg.NLOC, np.float32)
            dself[:nreal] = dinv[lo_n:hi_n]
            lanes = np.arange(cfg.NLOC)
            diagq_all[c, lanes % 128, (lanes // 128) * 128 + lanes % 128] = \
                dself.astype(bf)
        hops.append(dict(T=T, TL=TL, TH=TH, Ttot=Ttot, tile_w=tile_w,
                         seg_first=seg_first, seg_last=seg_last,
                         idx=idx_all, Q=Q_all, diagq=diagq_all, dok=dok_all))
    b = np.asarray(batch, np.int64)
    cnt = np.bincount(b, minlength=cfg.G)
    assert (cnt == cfg.NPG).all() and (np.sort(b) == b).all(), \
        "batch must be contiguous-uniform"
    key = tuple(int(h['Ttot']) for h in hops)
    return dict(hops=hops, key=key)


def build_bass(cfg, plan):
    import concourse.bacc as bacc
    import concourse.mybir as mybir
    from concourse.tile import TileContext
    from concourse.library_config import mlp as mlp_lib

    f32, bf16, i16 = mybir.dt.float32, mybir.dt.bfloat16, mybir.dt.int16
    Alu = mybir.AluOpType
    Act = mybir.ActivationFunctionType
    AX = mybir.AxisListType.X
    NC, L, W, CH = cfg.NCORES, cfg.L, cfg.W, cfg.CH
    HID = cfg.HID
    SM = cfg.SMAX
    RG = [list(range(NC))]

    nc = bacc.Bacc("TRN2", num_devices=NC, num_swdge_queues=4)

    xT = nc.dram_tensor("xT", [cfg.IN_DIM, cfg.NLOC], f32, kind="ExternalInput")
    idx_d, q_d, dq_d, dok_d = [], [], [], []
    for k in range(1, L + 1):
        hp = plan['hops'][k - 1]
        idx_d.append(nc.dram_tensor(f"idx{k}", [128, hp['Ttot'] * 8], i16,
                                    kind="ExternalInput"))
        q_d.append(nc.dram_tensor(f"q{k}", [128, hp['Ttot'] * 128], bf16,
                                  kind="ExternalInput"))
        dq_d.append(nc.dram_tensor(f"dq{k}", [128, W * 128], bf16,
                                   kind="ExternalInput"))
        dok_d.append(nc.dram_tensor(f"dok{k}", [128, W], f32,
                                    kind="ExternalInput"))
    ident_d = nc.dram_tensor("ident", [128, 128], f32, kind="ExternalInput")
    embWT_d = nc.dram_tensor("embWT", [cfg.IN_DIM, HID], f32, kind="ExternalInput")
    embB_d = nc.dram_tensor("embB", [1, HID], f32, kind="ExternalInput")
    convWT_d = nc.dram_tensor("convWT", [cfg.NCONV, HID, HID], bf16,
                              kind="ExternalInput")
    convB_d = nc.dram_tensor("convB", [cfg.NCONV, HID], f32, kind="ExternalInput")
    kinv_d = nc.dram_tensor("kinv", [cfg.NCONV, 1], f32, kind="ExternalInput")
    r1WT_d = nc.dram_tensor("r1WT", [3 * HID, 192], f32, kind="ExternalInput")
    r1B_d = nc.dram_tensor("r1B", [192, 1], f32, kind="ExternalInput")
    r2WT_d = nc.dram_tensor("r2WT", [192, cfg.OUT], f32, kind="ExternalInput")
    r2B_d = nc.dram_tensor("r2B", [cfg.OUT, 1], f32, kind="ExternalInput")
    y_d = nc.dram_tensor("y", [cfg.OUT, SM], f32, kind="ExternalOutput")

    tables = [nc.dram_tensor(f"tbl{j}", [cfg.TBL, HID], bf16, kind="Internal",
                             addr_space="Shared") for j in range(L)]
    hin = nc.dram_tensor("hin", [cfg.NLOC, HID], bf16, kind="Internal")
    ownblk = [nc.dram_tensor(f"own{j}", [128, W * HID], bf16, kind="Internal")
              for j in range(L)]

    hview = hin.rearrange("(w j) f -> j w f", j=128)

    with TileContext(nc) as tc:
        nc.gpsimd.load_library(mlp_lib)
        with tc.tile_pool(name="const", bufs=1) as constp, \
             tc.tile_pool(name="persist", bufs=1) as pers, \
             tc.tile_pool(name="io", bufs=2) as iop, \
             tc.tile_pool(name="own", bufs=1) as ownp, \
             tc.tile_pool(name="msg", bufs=5) as msgp, \
             tc.tile_pool(name="islab", bufs=5) as islabp, \
             tc.tile_pool(name="qstream", bufs=5) as qsp, \
             tc.tile_pool(name="qp", bufs=4) as qp, \
             tc.tile_pool(name="agg", bufs=3, space="PSUM") as aggp, \
             tc.tile_pool(name="outp", bufs=2, space="PSUM") as outp, \
             tc.tile_pool(name="smallps", bufs=2, space="PSUM") as smallp:

            ident = constp.tile([128, 128], f32)
            nc.sync.dma_start(ident[:], ident_d[:])
            ones_row = constp.tile([1, 128], f32)
            nc.vector.memset(ones_row[:], 1.0)

            xk = pers.tile([128, W, HID], f32)
            dok = [pers.tile([128, W], f32, tag=f"dok{k}", name=f"dok{k}")
                   for k in range(L)]
            h5T = pers.tile([128, W, 128], bf16)
            for k in range(L):
                nc.sync.dma_start(dok[k][:], dok_d[k][:])

            # ---- Phase A: h0 = x @ embW^T + emb_b
            embWT = constp.tile([cfg.IN_DIM, HID], f32)
            nc.sync.dma_start(embWT[:], embWT_d[:])
            embB = constp.tile([1, HID], f32)
            nc.sync.dma_start(embB[:], embB_d[:])
            xTs = pers.tile([cfg.IN_DIM, cfg.NLOC], f32)
            nc.sync.dma_start(xTs[:], xT[:])
            bias_ps = smallp.tile([128, 128], f32, tag="smallt")
            nc.tensor.matmul(bias_ps[:], ones_row[:], embB[:])
            embB_bc = constp.tile([128, 128], f32)
            nc.vector.tensor_copy(embB_bc[:], bias_ps[:])
            h0bf = pers.tile([128, W, HID], bf16, tag="hstage")
            for w in range(W):
                hps = outp.tile([128, HID], f32, tag="ops")
                nc.tensor.matmul(hps[:], xTs[:, w * 128:(w + 1) * 128], embWT[:])
                nc.vector.tensor_tensor(h0bf[:, w, :], hps[:], embB_bc[:], Alu.add)
            nc.sync.dma_start(hview[:, :, :], h0bf[:])
            nc.sync.dma_start(ownblk[0][:],
                              h0bf[:].rearrange("p w f -> p (w f)"))
            nc.gpsimd.collective_compute("AllGather", Alu.bypass,
                                         replica_groups=RG,
                                         ins=[hin[:]], outs=[tables[0][:]])

            qcycle = [0]

            def gather_stream(k, table, on_chunk):
                hp = plan['hops'][k - 1]
                for (h, lim0, lim1) in ((0, 0, hp['TL']),
                                        (1, hp['TL'], hp['Ttot'])):
                    t0 = lim0
                    while t0 < lim1:
                        n_t = min(CH, lim1 - t0)
                        islab = islabp.tile([128, CH * 8], i16, tag="islab")
                        nc.sync.dma_start(islab[:, :n_t * 8],
                                          idx_d[k - 1][:, t0 * 8:(t0 + n_t) * 8])
                        buf = msgp.tile([128, CH, HID], bf16, tag="gbuf")
                        tv = table.rearrange("(r two) f -> r (two f)", two=2)
                        src = tv[:, 0:HID] if h == 0 else tv[:, HID:2 * HID]
                        nc.gpsimd.dma_gather(buf[:, 0:n_t, :], src,
                                             islab[:, :n_t * 8],
                                             n_t * 128, n_t * 128, HID,
                                             elem_step=2 * HID,
                                             single_packet=False,
                                             queue_num=qcycle[0] % 4)
                        qcycle[0] += 1
                        on_chunk(t0, n_t, buf)
                        t0 += n_t

            # ---- Phase D: layers
            for l in range(L):
                nc.vector.memset(xk[:], 0.0)
                for k in range(l + 1, 0, -1):
                    ci = l * (l + 1) // 2 + (k - 1)
                    hp = plan['hops'][k - 1]
                    j = l - k + 1
                    tbl_j = tables[j]
                    wt = iop.tile([128, HID], bf16, tag="wt")
                    nc.sync.dma_start(wt[:], convWT_d[ci, :, :])
                    # self-loop term: per-window dense matmul vs scaled diag
                    ownb = ownp.tile([128, W, HID], bf16, tag="ownb")
                    nc.sync.dma_start(ownb[:],
                                      ownblk[j].rearrange("p (w f) -> p w f",
                                                          f=HID))
                    dq = ownp.tile([128, W * 128], bf16, tag="dq")
                    nc.sync.dma_start(dq[:], dq_d[k - 1][:])
                    for w in range(W):
                        sagg = aggp.tile([128, 128], f32, tag="agg",
                                         name="saggt")
                        nc.tensor.matmul(sagg[:], ownb[:, w, :],
                                         dq[:, w * 128:(w + 1) * 128],
                                         start=True, stop=True)
                        at2 = qp.tile([128, 128], bf16, tag="at")
                        nc.scalar.copy(at2[:], sagg[:])
                        ops2 = outp.tile([128, 128], f32, tag="ops")
                        nc.tensor.matmul(ops2[:], at2[:], wt[:])
                        nc.vector.scalar_tensor_tensor(
                            xk[:, w, :], ops2[:], dok[k - 1][:, w:w + 1],
                            xk[:, w, :], Alu.mult, Alu.add)
                    state = {}

                    def d_chunk(t0, n_t, buf, k=k, hp=hp, wt=wt, state=state):
                        qbuf = qsp.tile([128, CH * 128], bf16, tag="qb")
                        nc.sync.dma_start(qbuf[:, :n_t * 128],
                                          q_d[k - 1][:, t0 * 128:(t0 + n_t) * 128])
                        for i in range(n_t):
                            t = t0 + i
                            w = int(hp['tile_w'][t])
                            if hp['seg_first'][t]:
                                state['agg'] = aggp.tile(
                                    [128, 128], f32, tag="agg", name="aggt")
                            nc.tensor.matmul(state['agg'][:], buf[:, i, :],
                                             qbuf[:, i * 128:(i + 1) * 128],
                                             start=bool(hp['seg_first'][t]),
                                             stop=bool(hp['seg_last'][t]))
                            if hp['seg_last'][t]:
                                at = qp.tile([128, 128], bf16, tag="at")
                                nc.scalar.copy(at[:], state['agg'][:])
                                ops = outp.tile([128, 128], f32, tag="ops")
                                nc.tensor.matmul(ops[:], at[:], wt[:])
                                nc.vector.scalar_tensor_tensor(
                                    xk[:, w, :], ops[:], dok[k - 1][:, w:w + 1],
                                    xk[:, w, :], Alu.mult, Alu.add)
                    gather_stream(k, tbl_j, d_chunk)
                # bias: xk += bcast(sum_ci convB[ci] / k)
                c0 = l * (l + 1) // 2
                bsc = iop.tile([l + 1, HID], f32, tag="bsc")
                nc.sync.dma_start(bsc[:], convB_d[c0:c0 + l + 1, :])
                kv = iop.tile([l + 1, 1], f32, tag="kv")
                nc.sync.dma_start(kv[:], kinv_d[c0:c0 + l + 1, :])
                bscl = iop.tile([l + 1, HID], f32, tag="bscl")
                nc.vector.tensor_scalar_mul(bscl[:], bsc[:], kv[:, 0:1])
                ones_col = iop.tile([l + 1, 1], f32, tag="onescol")
                nc.vector.memset(ones_col[:], 1.0)
                brow_ps = smallp.tile([1, HID], f32, tag="smallt")
                nc.tensor.matmul(brow_ps[:], ones_col[:], bscl[:])
                brow = iop.tile([1, HID], f32, tag="brows")
                nc.vector.tensor_copy(brow[:], brow_ps[:])
                bbc_ps = smallp.tile([128, HID], f32, tag="smallt")
                nc.tensor.matmul(bbc_ps[:], ones_row[:], brow[:])
                bbc = iop.tile([128, HID], f32, tag="bbcs")
                nc.vector.tensor_copy(bbc[:], bbc_ps[:])
                nc.vector.tensor_tensor(xk[:], xk[:],
                                        bbc[:, None, :].broadcast_to(
                                            [128, W, HID]), Alu.add)
                if l < L - 1:
                    hbf = pers.tile([128, W, HID], bf16, tag="hstage")
                    nc.scalar.activation(hbf[:], xk[:], Act.Relu)
                    nc.sync.dma_start(hview[:, :, :], hbf[:])
                    nc.sync.dma_start(ownblk[l + 1][:],
                                      hbf[:].rearrange("p w f -> p (w f)"))
                    nc.gpsimd.collective_compute(
                        "AllGather", Alu.bypass, replica_groups=RG,
                        ins=[hin[:]], outs=[tables[l + 1][:]])
                else:
                    nc.scalar.activation(xk[:], xk[:], Act.Relu)

            # ---- Phase E: core-local pooling (batch is graph-aligned)
            for w in range(W):
                tp = outp.tile([128, 128], f32, tag="ops")
                nc.tensor.transpose(tp[:], xk[:, w, :], ident[:])
                nc.vector.tensor_copy(h5T[:, w, :], tp[:])
            h5flat = h5T[:].rearrange("p w j -> p (w j)")
            ssum_l = iop.tile([128, SM], f32, tag="ssum_l")
            smax_l = iop.tile([128, SM], f32, tag="smax_l")
            nc.vector.memset(ssum_l[:], 0.0)
            nc.vector.memset(smax_l[:], 0.0)
            for s in range(cfg.SMAXR):
                sl = h5flat[:, s * cfg.NPG:(s + 1) * cfg.NPG]
                nc.vector.tensor_reduce(ssum_l[:, s:s + 1], sl, axis=AX,
                                        op=Alu.add)
                nc.vector.tensor_reduce(smax_l[:, s:s + 1], sl, axis=AX,
                                        op=Alu.max)
            smean_l = iop.tile([128, SM], f32, tag="smean_l")
            nc.vector.tensor_scalar_mul(smean_l[:], ssum_l[:], 1.0 / cfg.NPG)

            # ---- Phase F: core-local MLP on [*, SMAX] graphs
            r1WTs = [constp.tile([HID, 192], f32, name=f"r1w{j}")
                     for j in range(3)]
            for j in range(3):
                nc.sync.dma_start(r1WTs[j][:], r1WT_d[j * HID:(j + 1) * HID, :])
            r1Bs = [constp.tile([128, 1], f32, name="r1b0"),
                    constp.tile([64, 1], f32, name="r1b1")]
            nc.sync.dma_start(r1Bs[0][:], r1B_d[0:128, :])
            nc.sync.dma_start(r1Bs[1][:], r1B_d[128:192, :])
            r2WTs = [constp.tile([128, cfg.OUT], f32, name="r2w0"),
                     constp.tile([64, cfg.OUT], f32, name="r2w1")]
            nc.sync.dma_start(r2WTs[0][:], r2WT_d[0:128, :])
            nc.sync.dma_start(r2WTs[1][:], r2WT_d[128:192, :])
            r2Bs = constp.tile([cfg.OUT, 1], f32)
            nc.sync.dma_start(r2Bs[:], r2B_d[:])
            chunks = (ssum_l, smax_l, smean_l)
            hidT = []
            for mi, (m0, m1) in enumerate(((0, 128), (128, 192))):
                hps2 = outp.tile([m1 - m0, SM], f32, tag="ops", name="hps2")
                for j in range(3):
                    nc.tensor.matmul(hps2[:], r1WTs[j][:, m0:m1],
                                     chunks[j][:], start=(j == 0), stop=(j == 2))
                hsb = iop.tile([m1 - m0, SM], f32, tag=f"hsb{m0}",
                               name=f"hsb{m0}")
                nc.scalar.activation(hsb[:], hps2[:], Act.Lrelu,
                                     bias=r1Bs[mi][:, 0:1], alpha=0.01)
                hidT.append(hsb)
            yps = outp.tile([cfg.OUT, SM], f32, tag="ops")
            nc.tensor.matmul(yps[:], r2WTs[0][:], hidT[0][:],
                             start=True, stop=False)
            nc.tensor.matmul(yps[:], r2WTs[1][:], hidT[1][:],
                             start=False, stop=True)
            ysb = iop.tile([cfg.OUT, SM], f32, tag="ysb")
            nc.scalar.activation(ysb[:], yps[:], Act.Identity,
                                 bias=r2Bs[:, 0:1])
            nc.sync.dma_start(y_d[:], ysb[:])

    nc.compile()
    return nc


def _wrap_idx(arr):
    w16 = arr.reshape(-1, 16).T
    return np.tile(w16, (8, 1)).copy()


def make_inmaps(cfg, plan, inputs):
    import ml_dtypes
    bf = ml_dtypes.bfloat16
    x = np.asarray(inputs['x'], np.float32)
    kinv = np.array([[1.0 / k] for l in range(cfg.L) for k in range(1, l + 2)],
                    np.float32)
    shared = dict(
        ident=np.eye(128, dtype=np.float32),
        embWT=np.ascontiguousarray(np.asarray(inputs['emb_W'], np.float32).T),
        embB=np.asarray(inputs['emb_b'], np.float32)[None, :].copy(),
        convWT=np.ascontiguousarray(
            np.asarray(inputs['conv_W'], np.float32).transpose(0, 2, 1)).astype(bf),
        convB=np.asarray(inputs['conv_b'], np.float32),
        kinv=kinv,
        r1WT=np.ascontiguousarray(np.asarray(inputs['r1_W'], np.float32).T),
        r1B=np.asarray(inputs['r1_b'], np.float32)[:, None].copy(),
        r2WT=np.ascontiguousarray(np.asarray(inputs['r2_W'], np.float32).T),
        r2B=np.asarray(inputs['r2_b'], np.float32)[:, None].copy(),
    )
    in_maps = []
    for c in range(cfg.NCORES):
        m = dict(shared)
        lo_n, hi_n = cfg.core_lo(c), cfg.core_hi(c)
        xs = np.zeros((cfg.NLOC, cfg.IN_DIM), np.float32)
        xs[:hi_n - lo_n] = x[lo_n:hi_n]
        m['xT'] = np.ascontiguousarray(xs.T)
        for k in range(1, cfg.L + 1):
            hp = plan['hops'][k - 1]
            m[f'idx{k}'] = _wrap_idx(hp['idx'][c])
            m[f'q{k}'] = hp['Q'][c]
            m[f'dq{k}'] = hp['diagq'][c]
            m[f'dok{k}'] = hp['dok'][c]
        in_maps.append(m)
    return in_maps


_CACHE = {}


def kernel(**inputs):
    from concourse.bass_utils import run_bass_kernel_spmd
    cfg = Cfg()
    plan = build_plan(cfg, np.asarray(inputs['k_edge_index']),
                      np.asarray(inputs['batch']))
    if plan['key'] not in _CACHE:
        _CACHE[plan['key']] = build_bass(cfg, plan)
    nc = _CACHE[plan['key']]
    in_maps = make_inmaps(cfg, plan, inputs)
    res = run_bass_kernel_spmd(nc, in_maps, core_ids=list(range(cfg.NCORES)))
    parts = []
    for c in range(cfg.NCORES):
        ng = (cfg.core_hi(c) - cfg.core_lo(c)) // cfg.NPG
        yc = np.asarray(res.results[c]['y'], np.float32)
        parts.append(yc[:, :ng].T)
    return np.ascontiguousarray(np.concatenate(parts, axis=0))


# revision 7
# speedup vs baseline: 2.0322x; 1.1209x over previous
"""DRew-GCN forward on 8 Trainium2 NeuronCores.

Node-partitioned across cores (block-contiguous, graph-aligned). Edges live
with the core owning dst; per hop they are grouped into 128-node dst windows
(uniform tile capacities across cores so one SPMD program serves all 8).
Scatter-add = one-hot matmul into PSUM. The one-hot scatter matrices Q
(Q[e,j] = dinv_k[src_e] * (dstloc[e]==j)) are fully data-dependent host
constants: prebuilt per hop (bf16) and streamed from DRAM, so no on-device
degree/coef computation or Q construction. Self-loops are excluded from the
edge stream; the self term is a per-window dense matmul against a host-built
scaled diagonal. Rows are fetched by dma_gather from replicated bf16 node
tables refreshed per layer with an AllGather. The GCN projection W is applied
after aggregation (linearity), so gathers move only raw 128-dim rows.
Pooling + MLP are core-local (batch is graph-aligned); the host concatenates
the per-core outputs, so no AllReduce is needed.
"""
import sys
import numpy as np

if '/opt/trn_rl_repo' not in sys.path:
    sys.path.insert(0, '/opt/trn_rl_repo')


class Cfg:
    def __init__(self, N=50000, G=500, IN_DIM=32, HID=128, OUT=10,
                 L=5, E_K=400000, NCORES=8, CH=32):
        self.N, self.G, self.IN_DIM, self.HID, self.OUT = N, G, IN_DIM, HID, OUT
        self.L, self.E_K, self.NCORES, self.CH = L, E_K, NCORES, CH
        self.NPG = N // G                       # nodes per graph
        assert N % G == 0
        self.GPC = -(-G // NCORES)              # graphs per core (ceil)
        self.BLK = self.GPC * self.NPG          # real nodes per core (last less)
        self.NLOC = 128 * (-(-self.BLK // 128))
        self.W = self.NLOC // 128
        self.TBL = NCORES * self.NLOC
        self.HALF = self.TBL // 2
        assert self.HALF < 32768
        self.NCONV = L * (L + 1) // 2
        self.SMAX = 64                          # pooling slice tile width
        self.SMAXR = self.GPC                   # real slices per core
        assert self.SMAXR <= self.SMAX

    def core_lo(self, c): return min(self.BLK * c, self.N)
    def core_hi(self, c): return min(self.BLK * (c + 1), self.N)

    def row_of(self, n):
        c = np.minimum(n // self.BLK, self.NCORES - 1)
        return self.NLOC * c + (n - self.BLK * c)


def build_plan(cfg, k_edge_index, batch):
    import ml_dtypes
    bf = ml_dtypes.bfloat16
    NC = cfg.NCORES
    hops = []
    for k in range(1, cfg.L + 1):
        src = np.asarray(k_edge_index[0, (k - 1) * cfg.E_K: k * cfg.E_K], np.int64)
        dst = np.asarray(k_edge_index[1, (k - 1) * cfg.E_K: k * cfg.E_K], np.int64)
        # symmetric-norm degree (in-degree + self-loop), full graph
        deg = np.bincount(dst, minlength=cfg.N).astype(np.float32) + 1.0
        dinv = 1.0 / np.sqrt(deg)
        per_core = []
        for c in range(NC):
            lo_n, hi_n = cfg.core_lo(c), cfg.core_hi(c)
            m = (dst >= lo_n) & (dst < hi_n)
            es, ed = src[m], dst[m]
            erow = cfg.row_of(es)
            dloc = ed - lo_n
            half = (erow % 2).astype(np.int64)
            per_core.append((es, erow, dloc, half, dloc // 128))
        T = np.zeros((cfg.W, 2), np.int64)
        for c in range(NC):
            _, _, _, half, w = per_core[c]
            for h in (0, 1):
                cnt = np.bincount(w[half == h], minlength=cfg.W)
                T[:, h] = np.maximum(T[:, h], -(-cnt // 128))
        TL, TH = int(T[:, 0].sum()), int(T[:, 1].sum())
        Ttot = TL + TH
        tile_w = np.concatenate([np.repeat(np.arange(cfg.W), T[:, 0]),
                                 np.repeat(np.arange(cfg.W), T[:, 1])])
        seg_first = np.zeros(Ttot, bool); seg_last = np.zeros(Ttot, bool)
        pos = 0
        seg_off = np.zeros((cfg.W, 2), np.int64)
        for h in (0, 1):
            for w in range(cfg.W):
                seg_off[w, h] = pos
                if T[w, h]:
                    seg_first[pos] = True
                    seg_last[pos + T[w, h] - 1] = True
                    pos += int(T[w, h])
        idx_all = np.zeros((NC, Ttot * 128), np.int16)
        Q_all = np.zeros((NC, 128, Ttot * 128), bf)
        diagq_all = np.zeros((NC, 128, cfg.W * 128), bf)
        dok_all = np.zeros((NC, 128, cfg.W), np.float32)
        for c in range(NC):
            es, erow, dloc, half, w = per_core[c]
            for h in (0, 1):
                sel = half == h
                ws, rows, dl, sn = w[sel], erow[sel], dloc[sel], es[sel]
                order = np.argsort(ws, kind='stable')
                ws, rows, dl, sn = ws[order], rows[order], dl[order], sn[order]
                cnts = np.bincount(ws, minlength=cfg.W)
                starts = seg_off[:, h] * 128
                grp0 = np.concatenate([[0], np.cumsum(cnts)[:-1]])
                pos_in = np.arange(len(ws)) - np.repeat(grp0, cnts)
                p = starts[ws] + pos_in
                idx_all[c, p] = (rows >> 1).astype(np.int16)
                Q_all[c, p % 128, (p // 128) * 128 + (dl - 128 * ws)] = \
                    dinv[sn].astype(bf)
            lo_n, hi_n = cfg.core_lo(c), cfg.core_hi(c)
            nreal = hi_n - lo_n
            dv = np.zeros(cfg.NLOC, np.float32)
            dv[:nreal] = dinv[lo_n:hi_n] / k
            dok_all[c] = dv.reshape(cfg.W, 128).T
            dself = np.zeros(cfg.NLOC, np.float32)
            dself[:nreal] = dinv[lo_n:hi_n]
            lanes = np.arange(cfg.NLOC)
            diagq_all[c, lanes % 128, (lanes // 128) * 128 + lanes % 128] = \
                dself.astype(bf)
        hops.append(dict(T=T, TL=TL, TH=TH, Ttot=Ttot, tile_w=tile_w,
                         seg_first=seg_first, seg_last=seg_last,
                         idx=idx_all, Q=Q_all, diagq=diagq_all, dok=dok_all))
    b = np.asarray(batch, np.int64)
    cnt = np.bincount(b, minlength=cfg.G)
    assert (cnt == cfg.NPG).all() and (np.sort(b) == b).all(), \
        "batch must be contiguous-uniform"
    key = tuple(int(h['Ttot']) for h in hops)
    return dict(hops=hops, key=key)


def build_bass(cfg, plan):
    import concourse.bacc as bacc
    import concourse.mybir as mybir
    from concourse.tile import TileContext
    from concourse.library_config import mlp as mlp_lib

    f32, bf16, i16 = mybir.dt.float32, mybir.dt.bfloat16, mybir.dt.int16
    Alu = mybir.AluOpType
    Act = mybir.ActivationFunctionType
    AX = mybir.AxisListType.X
    NC, L, W, CH = cfg.NCORES, cfg.L, cfg.W, cfg.CH
    HID = cfg.HID
    SM = cfg.SMAX
    RG = [list(range(NC))]

    nc = bacc.Bacc("TRN2", num_devices=NC, num_swdge_queues=4)

    xT = nc.dram_tensor("xT", [cfg.IN_DIM, cfg.NLOC], f32, kind="ExternalInput")
    idx_d, q_d, dq_d, dok_d = [], [], [], []
    for k in range(1, L + 1):
        hp = plan['hops'][k - 1]
        idx_d.append(nc.dram_tensor(f"idx{k}", [128, hp['Ttot'] * 8], i16,
                                    kind="ExternalInput"))
        q_d.append(nc.dram_tensor(f"q{k}", [128, hp['Ttot'] * 128], bf16,
                                  kind="ExternalInput"))
        dq_d.append(nc.dram_tensor(f"dq{k}", [128, W * 128], bf16,
                                   kind="ExternalInput"))
        dok_d.append(nc.dram_tensor(f"dok{k}", [128, W], f32,
                                    kind="ExternalInput"))
    ident_d = nc.dram_tensor("ident", [128, 128], f32, kind="ExternalInput")
    embWT_d = nc.dram_tensor("embWT", [cfg.IN_DIM, HID], f32, kind="ExternalInput")
    embB_d = nc.dram_tensor("embB", [1, HID], f32, kind="ExternalInput")
    convWT_d = nc.dram_tensor("convWT", [cfg.NCONV, HID, HID], bf16,
                              kind="ExternalInput")
    convB_d = nc.dram_tensor("convB", [cfg.NCONV, HID], f32, kind="ExternalInput")
    kinv_d = nc.dram_tensor("kinv", [cfg.NCONV, 1], f32, kind="ExternalInput")
    r1WT_d = nc.dram_tensor("r1WT", [3 * HID, 192], f32, kind="ExternalInput")
    r1B_d = nc.dram_tensor("r1B", [192, 1], f32, kind="ExternalInput")
    r2WT_d = nc.dram_tensor("r2WT", [192, cfg.OUT], f32, kind="ExternalInput")
    r2B_d = nc.dram_tensor("r2B", [cfg.OUT, 1], f32, kind="ExternalInput")
    y_d = nc.dram_tensor("y", [cfg.OUT, SM], f32, kind="ExternalOutput")

    tables = [nc.dram_tensor(f"tbl{j}", [cfg.TBL, HID], bf16, kind="Internal",
                             addr_space="Shared") for j in range(L)]
    hin = nc.dram_tensor("hin", [cfg.NLOC, HID], bf16, kind="Internal")
    ownblk = [nc.dram_tensor(f"own{j}", [128, W * HID], bf16, kind="Internal")
              for j in range(L)]

    hview = hin.rearrange("(w j) f -> j w f", j=128)

    with TileContext(nc) as tc:
        nc.gpsimd.load_library(mlp_lib)
        with tc.tile_pool(name="const", bufs=1) as constp, \
             tc.tile_pool(name="persist", bufs=1) as pers, \
             tc.tile_pool(name="io", bufs=2) as iop, \
             tc.tile_pool(name="own", bufs=1) as ownp, \
             tc.tile_pool(name="msg", bufs=5) as msgp, \
             tc.tile_pool(name="islab", bufs=5) as islabp, \
             tc.tile_pool(name="qstream", bufs=5) as qsp, \
             tc.tile_pool(name="qp", bufs=4) as qp, \
             tc.tile_pool(name="agg", bufs=3, space="PSUM") as aggp, \
             tc.tile_pool(name="outp", bufs=2, space="PSUM") as outp, \
             tc.tile_pool(name="smallps", bufs=2, space="PSUM") as smallp:

            ident = constp.tile([128, 128], f32)
            nc.sync.dma_start(ident[:], ident_d[:])
            ones_row = constp.tile([1, 128], f32)
            nc.vector.memset(ones_row[:], 1.0)

            xk = pers.tile([128, W, HID], f32)
            dok = [pers.tile([128, W], f32, tag=f"dok{k}", name=f"dok{k}")
                   for k in range(L)]
            h5T = pers.tile([128, W, 128], bf16)
            for k in range(L):
                nc.sync.dma_start(dok[k][:], dok_d[k][:])

            # ---- Phase A: h0 = x @ embW^T + emb_b
            embWT = constp.tile([cfg.IN_DIM, HID], f32)
            nc.sync.dma_start(embWT[:], embWT_d[:])
            embB = constp.tile([1, HID], f32)
            nc.sync.dma_start(embB[:], embB_d[:])
            xTs = pers.tile([cfg.IN_DIM, cfg.NLOC], f32)
            nc.sync.dma_start(xTs[:], xT[:])
            bias_ps = smallp.tile([128, 128], f32, tag="smallt")
            nc.tensor.matmul(bias_ps[:], ones_row[:], embB[:])
            embB_bc = constp.tile([128, 128], f32)
            nc.vector.tensor_copy(embB_bc[:], bias_ps[:])
            h0bf = pers.tile([128, W, HID], bf16, tag="hstage")
            for w in range(W):
                hps = outp.tile([128, HID], f32, tag="ops")
                nc.tensor.matmul(hps[:], xTs[:, w * 128:(w + 1) * 128], embWT[:])
                nc.vector.tensor_tensor(h0bf[:, w, :], hps[:], embB_bc[:], Alu.add)
            nc.sync.dma_start(hview[:, :, :], h0bf[:])
            nc.sync.dma_start(ownblk[0][:],
                              h0bf[:].rearrange("p w f -> p (w f)"))
            nc.gpsimd.collective_compute("AllGather", Alu.bypass,
                                         replica_groups=RG,
                                         ins=[hin[:]], outs=[tables[0][:]])

            qcycle = [0]

            def gather_stream(k, table, on_chunk):
                hp = plan['hops'][k - 1]
                for (h, lim0, lim1) in ((0, 0, hp['TL']),
                                        (1, hp['TL'], hp['Ttot'])):
                    t0 = lim0
                    while t0 < lim1:
                        n_t = min(CH, lim1 - t0)
                        islab = islabp.tile([128, CH * 8], i16, tag="islab")
                        nc.sync.dma_start(islab[:, :n_t * 8],
                                          idx_d[k - 1][:, t0 * 8:(t0 + n_t) * 8])
                        buf = msgp.tile([128, CH, HID], bf16, tag="gbuf")
                        tv = table.rearrange("(r two) f -> r (two f)", two=2)
                        src = tv[:, 0:HID] if h == 0 else tv[:, HID:2 * HID]
                        nc.gpsimd.dma_gather(buf[:, 0:n_t, :], src,
                                             islab[:, :n_t * 8],
                                             n_t * 128, n_t * 128, HID,
                                             elem_step=2 * HID,
                                             single_packet=False,
                                             queue_num=qcycle[0] % 4)
                        qcycle[0] += 1
                        on_chunk(t0, n_t, buf)
                        t0 += n_t

            # ---- Phase D: layers
            for l in range(L):
                nc.vector.memset(xk[:], 0.0)
                for k in range(l + 1, 0, -1):
                    ci = l * (l + 1) // 2 + (k - 1)
                    hp = plan['hops'][k - 1]
                    j = l - k + 1
                    tbl_j = tables[j]
                    wt = iop.tile([128, HID], bf16, tag="wt")
                    nc.sync.dma_start(wt[:], convWT_d[ci, :, :])
                    # self-loop term: per-window dense matmul vs scaled diag
                    ownb = ownp.tile([128, W, HID], bf16, tag="ownb")
                    nc.sync.dma_start(ownb[:],
                                      ownblk[j].rearrange("p (w f) -> p w f",
                                                          f=HID))
                    dq = ownp.tile([128, W * 128], bf16, tag="dq")
                    nc.sync.dma_start(dq[:], dq_d[k - 1][:])
                    for w in range(W):
                        sagg = aggp.tile([128, 128], f32, tag="agg",
                                         name="saggt")
                        nc.tensor.matmul(sagg[:], ownb[:, w, :],
                                         dq[:, w * 128:(w + 1) * 128],
                                         start=True, stop=True)
                        at2 = qp.tile([128, 128], bf16, tag="at")
                        nc.scalar.copy(at2[:], sagg[:])
                        ops2 = outp.tile([128, 128], f32, tag="ops")
                        nc.tensor.matmul(ops2[:], at2[:], wt[:])
                        nc.vector.scalar_tensor_tensor(
                            xk[:, w, :], ops2[:], dok[k - 1][:, w:w + 1],
                            xk[:, w, :], Alu.mult, Alu.add)
                    state = {}

                    def d_chunk(t0, n_t, buf, k=k, hp=hp, wt=wt, state=state):
                        qbuf = qsp.tile([128, CH * 128], bf16, tag="qb")
                        nc.sync.dma_start(qbuf[:, :n_t * 128],
                                          q_d[k - 1][:, t0 * 128:(t0 + n_t) * 128])
                        for i in range(n_t):
                            t = t0 + i
                            w = int(hp['tile_w'][t])
                            if hp['seg_first'][t]:
                                state['agg'] = aggp.tile(
                                    [128, 128], f32, tag="agg", name="aggt")
                            nc.tensor.matmul(state['agg'][:], buf[:, i, :],
                                             qbuf[:, i * 128:(i + 1) * 128],
                                             start=bool(hp['seg_first'][t]),
                                             stop=bool(hp['seg_last'][t]))
                            if hp['seg_last'][t]:
                                at = qp.tile([128, 128], bf16, tag="at")
                                nc.scalar.copy(at[:], state['agg'][:])
                                ops = outp.tile([128, 128], f32, tag="ops")
                                nc.tensor.matmul(ops[:], at[:], wt[:])
                                nc.vector.scalar_tensor_tensor(
                                    xk[:, w, :], ops[:], dok[k - 1][:, w:w + 1],
                                    xk[:, w, :], Alu.mult, Alu.add)
                    gather_stream(k, tbl_j, d_chunk)
                # bias: xk += bcast(sum_ci convB[ci] / k)
                c0 = l * (l + 1) // 2
                bsc = iop.tile([l + 1, HID], f32, tag="bsc")
                nc.sync.dma_start(bsc[:], convB_d[c0:c0 + l + 1, :])
                kv = iop.tile([l + 1, 1], f32, tag="kv")
                nc.sync.dma_start(kv[:], kinv_d[c0:c0 + l + 1, :])
                bscl = iop.tile([l + 1, HID], f32, tag="bscl")
                nc.vector.tensor_scalar_mul(bscl[:], bsc[:], kv[:, 0:1])
                ones_col = iop.tile([l + 1, 1], f32, tag="onescol")
                nc.vector.memset(ones_col[:], 1.0)
                brow_ps = smallp.tile([1, HID], f32, tag="smallt")
                nc.tensor.matmul(brow_ps[:], ones_col[:], bscl[:])
                brow = iop.tile([1, HID], f32, tag="brows")
                nc.vector.tensor_copy(brow[:], brow_ps[:])
                bbc_ps = smallp.tile([128, HID], f32, tag="smallt")
                nc.tensor.matmul(bbc_ps[:], ones_row[:], brow[:])
                bbc = iop.tile([128, HID], f32, tag="bbcs")
                nc.vector.tensor_copy(bbc[:], bbc_ps[:])
                nc.vector.tensor_tensor(xk[:], xk[:],
                                        bbc[:, None, :].broadcast_to(
                                            [128, W, HID]), Alu.add)
                if l < L - 1:
                    hbf = pers.tile([128, W, HID], bf16, tag="hstage")
                    nc.scalar.activation(hbf[:], xk[:], Act.Relu)
                    nc.sync.dma_start(hview[:, :, :], hbf[:])
                    nc.sync.dma_start(ownblk[l + 1][:],
                                      hbf[:].rearrange("p w f -> p (w f)"))
                    nc.gpsimd.collective_compute(
                        "AllGather", Alu.bypass, replica_groups=RG,
                        ins=[hin[:]], outs=[tables[l + 1][:]])
                else:
                    nc.scalar.activation(xk[:], xk[:], Act.Relu)

            # ---- Phase E: core-local pooling (batch is graph-aligned)
            for w in range(W):
                tp = outp.tile([128, 128], f32, tag="ops")
                nc.tensor.transpose(tp[:], xk[:, w, :], ident[:])
                nc.vector.tensor_copy(h5T[:, w, :], tp[:])
            h5flat = h5T[:].rearrange("p w j -> p (w j)")
            ssum_l = iop.tile([128, SM], f32, tag="ssum_l")
            smax_l = iop.tile([128, SM], f32, tag="smax_l")
            nc.vector.memset(ssum_l[:], 0.0)
            nc.vector.memset(smax_l[:], 0.0)
            for s in range(cfg.SMAXR):
                sl = h5flat[:, s * cfg.NPG:(s + 1) * cfg.NPG]
                nc.vector.tensor_reduce(ssum_l[:, s:s + 1], sl, axis=AX,
                                        op=Alu.add)
                nc.vector.tensor_reduce(smax_l[:, s:s + 1], sl, axis=AX,
                                        op=Alu.max)
            smean_l = iop.tile([128, SM], f32, tag="smean_l")
            nc.vector.tensor_scalar_mul(smean_l[:], ssum_l[:], 1.0 / cfg.NPG)

            # ---- Phase F: core-local MLP on [*, SMAX] graphs
            r1WTs = [constp.tile([HID, 192], f32, name=f"r1w{j}")
                     for j in range(3)]
            for j in range(3):
                nc.sync.dma_start(r1WTs[j][:], r1WT_d[j * HID:(j + 1) * HID, :])
            r1Bs = [constp.tile([128, 1], f32, name="r1b0"),
                    constp.tile([64, 1], f32, name="r1b1")]
            nc.sync.dma_start(r1Bs[0][:], r1B_d[0:128, :])
            nc.sync.dma_start(r1Bs[1][:], r1B_d[128:192, :])
            r2WTs = [constp.tile([128, cfg.OUT], f32, name="r2w0"),
                     constp.tile([64, cfg.OUT], f32, name="r2w1")]
            nc.sync.dma_start(r2WTs[0][:], r2WT_d[0:128, :])
            nc.sync.dma_start(r2WTs[1][:], r2WT_d[128:192, :])
            r2Bs = constp.tile([cfg.OUT, 1], f32)
            nc.sync.dma_start(r2Bs[:], r2B_d[:])
            chunks = (ssum_l, smax_l, smean_l)
            hidT = []
            for mi, (m0, m1) in enumerate(((0, 128), (128, 192))):
                hps2 = outp.tile([m1 - m0, SM], f32, tag="ops", name="hps2")
                for j in range(3):
                    nc.tensor.matmul(hps2[:], r1WTs[j][:, m0:m1],
                                     chunks[j][:], start=(j == 0), stop=(j == 2))
                hsb = iop.tile([m1 - m0, SM], f32, tag=f"hsb{m0}",
                               name=f"hsb{m0}")
                nc.scalar.activation(hsb[:], hps2[:], Act.Lrelu,
                                     bias=r1Bs[mi][:, 0:1], alpha=0.01)
                hidT.append(hsb)
            yps = outp.tile([cfg.OUT, SM], f32, tag="ops")
            nc.tensor.matmul(yps[:], r2WTs[0][:], hidT[0][:],
                             start=True, stop=False)
            nc.tensor.matmul(yps[:], r2WTs[1][:], hidT[1][:],
                             start=False, stop=True)
            ysb = iop.tile([cfg.OUT, SM], f32, tag="ysb")
            nc.scalar.activation(ysb[:], yps[:], Act.Identity,
                                 bias=r2Bs[:, 0:1])
            nc.sync.dma_start(y_d[:], ysb[:])

    nc.compile()
    return nc


def _wrap_idx(arr):
    w16 = arr.reshape(-1, 16).T
    return np.tile(w16, (8, 1)).copy()


def make_inmaps(cfg, plan, inputs):
    import ml_dtypes
    bf = ml_dtypes.bfloat16
    x = np.asarray(inputs['x'], np.float32)
    kinv = np.array([[1.0 / k] for l in range(cfg.L) for k in range(1, l + 2)],
                    np.float32)
    shared = dict(
        ident=np.eye(128, dtype=np.float32),
        embWT=np.ascontiguousarray(np.asarray(inputs['emb_W'], np.float32).T),
        embB=np.asarray(inputs['emb_b'], np.float32)[None, :].copy(),
        convWT=np.ascontiguousarray(
            np.asarray(inputs['conv_W'], np.float32).transpose(0, 2, 1)).astype(bf),
        convB=np.asarray(inputs['conv_b'], np.float32),
        kinv=kinv,
        r1WT=np.ascontiguousarray(np.asarray(inputs['r1_W'], np.float32).T),
        r1B=np.asarray(inputs['r1_b'], np.float32)[:, None].copy(),
        r2WT=np.ascontiguousarray(np.asarray(inputs['r2_W'], np.float32).T),
        r2B=np.asarray(inputs['r2_b'], np.float32)[:, None].copy(),
    )
    in_maps = []
    for c in range(cfg.NCORES):
        m = dict(shared)
        lo_n, hi_n = cfg.core_lo(c), cfg.core_hi(c)
        xs = np.zeros((cfg.NLOC, cfg.IN_DIM), np.float32)
        xs[:hi_n - lo_n] = x[lo_n:hi_n]
        m['xT'] = np.ascontiguousarray(xs.T)
        for k in range(1, cfg.L + 1):
            hp = plan['hops'][k - 1]
            m[f'idx{k}'] = _wrap_idx(hp['idx'][c])
            m[f'q{k}'] = hp['Q'][c]
            m[f'dq{k}'] = hp['diagq'][c]
            m[f'dok{k}'] = hp['dok'][c]
        in_maps.append(m)
    return in_maps


_CACHE = {}


def kernel(**inputs):
    from concourse.bass_utils import run_bass_kernel_spmd
    cfg = Cfg()
    plan = build_plan(cfg, np.asarray(inputs['k_edge_index']),
                      np.asarray(inputs['batch']))
    if plan['key'] not in _CACHE:
        _CACHE[plan['key']] = build_bass(cfg, plan)
    nc = _CACHE[plan['key']]
    in_maps = make_inmaps(cfg, plan, inputs)
    res = run_bass_kernel_spmd(nc, in_maps, core_ids=list(range(cfg.NCORES)))
    parts = []
    for c in range(cfg.NCORES):
        ng = (cfg.core_hi(c) - cfg.core_lo(c)) // cfg.NPG
        yc = np.asarray(res.results[c]['y'], np.float32)
        parts.append(yc[:, :ng].T)
    return np.ascontiguousarray(np.concatenate(parts, axis=0))
